# revision 35
# baseline (speedup 1.0000x reference)
"""KPConv (nn_KPConvFPN) Trainium2 Bass kernel — sparse candidate-pair design.

kw = relu(1 - |s[m] - q[n] - kp_p|/sigma) is ~97.6% zero for these inputs
(points uniform in [0,1]^3, sigma + max|kp| = 0.0825). The host finds a
conservative SUPERSET of candidate (query, neighbor) pairs by integer cell
binning (GRID=128; any pair within reach of any kernel point is provably
included; no float math decides output values on the host). Per core
(batch b=c//2, query half c%2):

Device pipeline (CAP=5120 candidate pairs, 40 blocks of 128):
  1. Per 1024-pair chunk: SWDGE dma_gather of combined 256B rows
     [64 f16 feats | s-coords f32] from ftab; PE-transpose feats -> fT;
     DVE/ACT compute kw[t, p] for all 15 kernel points.
     (query coords arrive pre-gathered from host as qsel, like the dense
     kernel's qrep.)
  2. Per 128-pair block: 4 PE matmuls fW = fT @ [W_0|..|W_15]/16 (f16,
     2048 psum cols); DVE multiply by kw broadcast over C_out; binary-tree
     add over the 16 p-slots -> ct[t, 128].
  3. Segment matmul psum2[d, o] = seg[t, d]^T @ ct (host-built 0/1 seg
     matrix; groups pairs of the same query; pads/unused -> trash slot),
     so every output row is scattered EXACTLY once (dma_scatter_add loses
     updates on duplicate rows -- measured).
  4. dma_scatter_add rows into the bias-prefilled donated output buffer
     (row 8192 = trash row for pad slots).

Falls back to the dense kernel (build_bass below) when candidates exceed
CAP. The reference divides by the count of neighbors with nonzero
features; for randn features that is always K=16 (folded into W/16); the
degenerate case is corrected exactly on the host.
"""
import json
import math
import os

SKIP = set()

import numpy as np
import jax

import concourse.bass as bass
import concourse.mybir as mybir
from concourse.tile import TileContext
from concourse import library_config
from concourse import bass2jax

F32 = mybir.dt.float32
F16 = mybir.dt.float16
I16 = mybir.dt.int16

B, N, M, K = 4, 16384, 16384, 16
C_IN, C_OUT, P = 64, 128, 15
SIGMA = 0.03
N_CORES = 8
NQ_CORE = N // 2            # 8192 queries per core
NK_CORE = NQ_CORE * K       # 131072 gathered rows per core
ST_Q = 512                  # queries per supertile
N_ST = NQ_CORE // ST_Q      # 16
KW_ST = 2                   # supertiles per kw group
G_ST = ST_Q * K // 128      # 64 g-cols per supertile
ROW16 = 128                 # fp16 units per table row (256B)

# ---------------------------------------------------------------------------
# walrus workaround: this nix walrus build supports ONE sync-wait per
# instruction; split extra waits onto NoOps inserted before the offender
# (same-engine program order preserves semantics). Also run
# codegen_inst_isa_subclasses (Bacc does; raw Bass doesn't) so extended
# instructions get their ISA bytes.
_orig_to_json_bytes = bass.Bass.to_json_bytes


def _fix_block(bb, ctr):
    insts = bb.get("instructions")
    if not isinstance(insts, list):
        return
    new = []
    for inst in insts:
        si = inst.get("sync_info")
        ow = si.get("on_wait") if isinstance(si, dict) else None
        if ow and len(ow) > 1:
            for w in ow[:-1]:
                ctr[0] += 1
                nop = {"engine": inst["engine"], "ins": [], "outs": [],
                       "name": f"I-wsplit-{ctr[0]}", "opcode": "NoOp",
                       "sync_info": {"on_update": [], "on_wait": [w]},
                       "text_hint": "wsplit"}
                if "debug" in inst:
                    nop["debug"] = inst["debug"]
                new.append(nop)
            si["on_wait"] = [ow[-1]]
        new.append(inst)
    bb["instructions"] = new


def _walk(o, ctr):
    if isinstance(o, dict):
        if isinstance(o.get("instructions"), list):
            _fix_block(o, ctr)
        for v in o.values():
            _walk(v, ctr)
    elif isinstance(o, list):
        for v in o:
            _walk(v, ctr)


def _to_json_bytes_split(self):
    mybir.codegen_inst_isa_subclasses(self)
    raw = _orig_to_json_bytes(self)
    d = json.loads(raw)
    ctr = [0]
    _walk(d, ctr)
    return json.dumps(d).encode()


bass.Bass.to_json_bytes = _to_json_bytes_split


def ap_view(t_ap, extra_offset, dims):
    """AP over tile t_ap with explicit free dims [[step, count], ...]
    (steps in elements); partition dim is taken from the tile."""
    return bass.AP(t_ap.tensor, t_ap.offset + extra_offset,
                   [t_ap.ap[0]] + list(dims))


def ap_part(t_ap, pstart, pcount, extra_offset, dims):
    pstep = t_ap.ap[0][0]
    return bass.AP(t_ap.tensor, t_ap.offset + pstart * pstep + extra_offset,
                   [[pstep, pcount]] + list(dims))


def build_bass(kp, reps=0, skip=()):
    global SKIP
    SKIP = set(skip)
    """kp: (15, 3) float32 numpy kernel points (runtime values baked)."""
    kpsq = (kp * kp).sum(axis=1)  # |kp_p|^2
    nc = bass.Bass(dynamic_dma_scratch_size=32768, num_swdge_queues=4)

    feats_in = nc.dram_tensor("sfeat", [M, C_IN], F32, kind="ExternalInput")
    pts_in = nc.dram_tensor("spts", [M, 3], F32, kind="ExternalInput")
    qrep_in = nc.dram_tensor("qrep", [128, NK_CORE // 128, 3], F32,
                             kind="ExternalInput")
    idx_in = nc.dram_tensor("idx", [128, NK_CORE // 16], I16,
                            kind="ExternalInput")
    w_in = nc.dram_tensor("w", [P, C_IN, C_OUT], F32, kind="ExternalInput")
    bias_in = nc.dram_tensor("bias", [C_OUT, 1], F32, kind="ExternalInput")
    mask120_in = nc.dram_tensor("mask120", [128, 120], F32, kind="ExternalInput")
    mask16_in = nc.dram_tensor("mask16", [128, 8], F32, kind="ExternalInput")
    ident_in = nc.dram_tensor("ident", [128, 128], F32, kind="ExternalInput")
    ones1_in = nc.dram_tensor("ones1", [1, 128], F32, kind="ExternalInput")
    kpb_in = nc.dram_tensor("kpb", [128, 48], F32, kind="ExternalInput")
    onesc_in = nc.dram_tensor("onesc", [128, 1], F32, kind="ExternalInput")
    out_t = nc.dram_tensor("out", [NQ_CORE, C_OUT], F32, kind="ExternalOutput")
    table = nc.dram_tensor("table", [M, ROW16], F16, kind="Internal")

    # library load as raw preamble (before Tile scheduling) so it is
    # guaranteed to precede every dma_gather on the Pool engine.
    nc.gpsimd.load_library(library_config.mlp)

    with TileContext(nc) as tc:
        with tc.tile_pool(name="const", bufs=1) as cpool, \
             tc.tile_pool(name="build", bufs=1) as bpool, \
             tc.tile_pool(name="gath", bufs=2) as gpool, \
             tc.tile_pool(name="kwp", bufs=2) as kwpool, \
             tc.tile_pool(name="kbd", bufs=1) as kbpool, \
             tc.tile_pool(name="wt", bufs=1) as wtpool, \
             tc.tile_pool(name="sm", bufs=3) as smpool, \
             tc.tile_pool(name="fin", bufs=2) as fpool, \
             tc.tile_pool(name="ps1", bufs=2, space="PSUM") as ps1pool, \
             tc.tile_pool(name="ps2", bufs=2, space="PSUM") as ps2pool, \
             tc.tile_pool(name="ps3", bufs=1, space="PSUM") as ps3pool:

            # ---- constants ----
            wp_t = cpool.tile([C_IN, P * C_OUT], F32, tag="wp")
            nc.sync.dma_start(
                wp_t[:].rearrange("c (p o) -> c p o", p=P),
                w_in[:].rearrange("p c o -> c p o"))
            bias_t = cpool.tile([C_OUT, 1], F32, tag="bias")
            nc.sync.dma_start(bias_t[:], bias_in[:])
            mask120_t = cpool.tile([128, 120], F32, tag="m120")
            nc.sync.dma_start(mask120_t[:], mask120_in[:])
            mask16_t = cpool.tile([128, 8], F32, tag="m16")
            nc.sync.dma_start(mask16_t[:], mask16_in[:])
            ident_t = cpool.tile([128, 128], F32, tag="ident")
            nc.sync.dma_start(ident_t[:], ident_in[:])
            ones1_t = cpool.tile([1, 128], F32, tag="ones1")
            nc.sync.dma_start(ones1_t[:], ones1_in[:])
            kpb_t = cpool.tile([128, 48], F32, tag="kpb")
            nc.sync.dma_start(kpb_t[:], kpb_in[:])
            onesc_t = cpool.tile([128, 1], F32, tag="onesc")
            nc.sync.dma_start(onesc_t[:], onesc_in[:])
            nidx_reg = nc.gpsimd.to_reg(1024)

            # ---- 1. combined table build (8 chunks x 2048 rows) ----
            import contextlib
            loop_cm = tc.For_i(0, reps, 1) if reps else contextlib.nullcontext()
            with loop_cm:
                _table_build(nc, tc, bpool, feats_in, pts_in, table)
                _main_pipeline(nc, tc, gpool, kwpool, kbpool, wtpool, smpool,
                               fpool, ps1pool, ps2pool, ps3pool, kp,
                               qrep_in, idx_in, out_t, table, wp_t, bias_t,
                               mask120_t, mask16_t, ident_t, ones1_t, kpb_t,
                               onesc_t, nidx_reg)
    return nc


def _table_build(nc, tc, bpool, feats_in, pts_in, table):
            for ch in range(8):
                m0 = ch * 2048
                fsb = bpool.tile([128, 16, C_IN], F32, tag="fsb")
                nc.sync.dma_start(
                    fsb[:],
                    feats_in[m0:m0 + 2048, :].rearrange(
                        "(a p) c -> p a c", p=128))
                psb = bpool.tile([128, 16, 3], F32, tag="psb")
                nc.sync.dma_start(
                    psb[:],
                    pts_in[m0:m0 + 2048, :].rearrange(
                        "(a p) c -> p a c", p=128))
                st16 = bpool.tile([128, 16, ROW16], F16, tag="st16")
                nc.vector.tensor_copy(st16[:, :, 0:C_IN], fsb[:])
                stv32 = st16[:].bitcast(F32)  # [128, 16, 64] f32 view
                # aux: sx sy sz at f32-cols 32..34
                nc.vector.tensor_copy(
                    bass.AP(stv32.tensor, stv32.offset + 32,
                            [stv32.ap[0], [64, 16], [1, 3]]),
                    psb[:])
                # |s|^2 at f32-col 35
                psq = bpool.tile([128, 16, 3], F32, tag="psq")
                nc.vector.tensor_tensor(out=psq[:], in0=psb[:], in1=psb[:],
                                        op=mybir.AluOpType.mult)
                nc.vector.tensor_reduce(
                    out=bass.AP(stv32.tensor, stv32.offset + 35,
                                [stv32.ap[0], [64, 16], [1, 1]]),
                    in_=psq[:], axis=mybir.AxisListType.X,
                    op=mybir.AluOpType.add)
                # z at f32-col 36: (sum_c |f|) > 0
                zred = bpool.tile([128, 16, 1], F32, tag="zred")
                nc.vector.tensor_reduce(out=zred[:], in_=fsb[:],
                                        axis=mybir.AxisListType.X,
                                        op=mybir.AluOpType.add,
                                        apply_absolute_value=True)
                nc.vector.tensor_scalar(
                    out=bass.AP(stv32.tensor, stv32.offset + 36,
                                [stv32.ap[0], [64, 16], [1, 1]]),
                    in0=zred[:], scalar1=0.0, scalar2=None,
                    op0=mybir.AluOpType.is_gt)
                nc.sync.dma_start(
                    table[m0:m0 + 2048, :].rearrange("(a p) c -> p a c",
                                                     p=128),
                    st16[:])


def _main_pipeline(nc, tc, gpool, kwpool, kbpool, wtpool, smpool, fpool,
                   ps1pool, ps2pool, ps3pool, kp, qrep_in, idx_in, out_t,
                   table, wp_t, bias_t, mask120_t, mask16_t, ident_t,
                   ones1_t, kpb_t, onesc_t, nidx_reg):
            for kg in range(N_ST // KW_ST):  # kw group of 2 supertiles
                GQ = KW_ST * ST_Q            # 1024 queries
                GG = KW_ST * G_ST            # 128 g-cols
                gt = gpool.tile([128, GG, ROW16], F16, tag="gath")
                gt32 = gt[:].bitcast(F32)  # [128, GG, 64] f32 view
                # gathers: 16 chunks of 1024 idx
                if "gather" in SKIP:
                    nc.vector.memset(gt[:], 0.0)
                for g in range(GG // 8):
                    if "gather" in SKIP:
                        break
                    idxsl = smpool.tile([128, 64], I16, tag="idxsl")
                    nc.sync.dma_start(
                        idxsl[:],
                        idx_in[:, (kg * 16 + g) * 64:(kg * 16 + g) * 64 + 64])
                    nc.gpsimd.dma_gather(
                        gt[:, g * 8:(g + 1) * 8, :], table[:], idxsl[:],
                        1024, nidx_reg, ROW16, queue_num=g % 4)
                # qrep slice
                qr = smpool.tile([128, GG, 3], F32, tag="qr")
                nc.sync.dma_start(qr[:], qrep_in[:, kg * GG:(kg + 1) * GG, :])
                # rel = s - q
                rel = smpool.tile([128, GG, 3], F32, tag="rel")
                nc.vector.tensor_tensor(
                    out=rel[:],
                    in0=ap_view(gt32, 32, [[64, GG], [1, 3]]),
                    in1=qr[:], op=mybir.AluOpType.subtract)
                # d2[p] = sum_dim (rel_dim - kp[p,dim])^2  (ACT squares, DVE adds)
                kwt = kwpool.tile([128, GG, P], F32, tag="kw")
                sq0 = smpool.tile([128, GG], F32, tag="sq0")
                sq1 = smpool.tile([128, GG], F32, tag="sq1")
                if "kw" in SKIP:
                    nc.vector.memset(kwt[:], 0.0)
                for p in range(P if "kw" not in SKIP else 0):
                    d2dst = ap_view(kwt[:], p, [[P, GG], [1, 1]])
                    nc.scalar.activation(
                        sq0[:], ap_view(rel[:], 0, [[3, GG], [1, 1]]),
                        mybir.ActivationFunctionType.Square,
                        bias=kpb_t[:, 3 * p:3 * p + 1], scale=1.0)
                    nc.scalar.activation(
                        sq1[:], ap_view(rel[:], 1, [[3, GG], [1, 1]]),
                        mybir.ActivationFunctionType.Square,
                        bias=kpb_t[:, 3 * p + 1:3 * p + 2], scale=1.0)
                    nc.vector.tensor_tensor(out=sq0[:], in0=sq0[:],
                                            in1=sq1[:],
                                            op=mybir.AluOpType.add)
                    nc.scalar.activation(
                        sq1[:], ap_view(rel[:], 2, [[3, GG], [1, 1]]),
                        mybir.ActivationFunctionType.Square,
                        bias=kpb_t[:, 3 * p + 2:3 * p + 3], scale=1.0)
                    nc.vector.tensor_tensor(out=d2dst, in0=sq0[:],
                                            in1=sq1[:],
                                            op=mybir.AluOpType.add)
                # kw = relu(1 - sqrt(d2 + 1e-10)/sigma), in place
                if "kw" in SKIP:
                    pass
                else:
                    nc.scalar.activation(kwt[:], kwt[:],
                                     mybir.ActivationFunctionType.Sqrt,
                                     bias=kpb_t[:, 45:46], scale=1.0)
                if "kw" not in SKIP:
                    nc.scalar.activation(kwt[:], kwt[:],
                                     mybir.ActivationFunctionType.Relu,
                                     bias=1.0, scale=kpb_t[:, 46:47])

                for sti in range(KW_ST):
                    st = kg * KW_ST + sti
                    # kwbd (2 half-ST TT ops): [128, (bl32, q8, p15)] fp16
                    kbd = kbpool.tile([128, 3840], F16, tag="kbd")
                    kbd2 = kbpool.tile([128, 3840], F16, tag="kbd2")
                    if "kwbd" in SKIP:
                        nc.vector.memset(kbd[:], 0.0)
                        nc.vector.memset(kbd2[:], 0.0)
                    for hf, kb in ((0, kbd), (1, kbd2)) if "kwbd" not in SKIP else ():
                        bl0 = sti * G_ST + hf * 32
                        nc.vector.tensor_tensor(
                            out=ap_view(kb[:], 0,
                                        [[120, 32], [15, 8], [1, 15]]),
                            in0=ap_view(kwt[:], bl0 * P,
                                        [[P, 32], [0, 8], [1, P]]),
                            in1=ap_view(mask120_t[:], 0,
                                        [[0, 32], [15, 8], [1, 15]]),
                            op=mybir.AluOpType.mult)
                    # einsum1: 64 blocks
                    wtt = wtpool.tile([64, 7680], F32, tag="wt")
                    if "e1" in SKIP:
                        nc.vector.memset(wtt[:], 0.0)
                    for bg in range(16 if "e1" not in SKIP else 0):  # bank groups of 4 blocks (32 q)
                        pse1 = ps1pool.tile([64, 480], F32, tag="pse1")
                        for j in range(4):
                            bl = bg * 4 + j          # block in supertile
                            blg = sti * G_ST + bl    # g-col in group tile
                            kb = kbd if bl < 32 else kbd2
                            kbl = bl % 32
                            nc.tensor.matmul(
                                pse1[:, j * 120:(j + 1) * 120],
                                ap_view(gt[:], blg * ROW16, [[1, C_IN]]),
                                ap_view(kb[:], kbl * 120, [[1, 120]]),
                                start=True, stop=True)
                        # evict (split DVE/ACT)
                        nc.vector.tensor_copy(
                            wtt[:, bg * 480:bg * 480 + 240],
                            pse1[:, 0:240])
                        nc.scalar.copy(
                            wtt[:, bg * 480 + 240:bg * 480 + 480],
                            pse1[:, 240:480])
                    # count row: zbd = z * mask16 -> ones-row matmul
                    zbd = smpool.tile([128, 512], F32, tag="zbd")
                    nc.vector.tensor_tensor(
                        out=zbd[:].rearrange("a (g j q) -> a g j q",
                                             g=16, j=4),
                        in0=ap_view(gt32, (sti * G_ST) * 64 + 36,
                                    [[256, 16], [64, 4], [0, 8]]),
                        in1=ap_view(mask16_t[:], 0,
                                    [[0, 16], [0, 4], [1, 8]]),
                        op=mybir.AluOpType.mult)
                    pscnt = ps3pool.tile([1, 512], F32, tag="pscnt")
                    nc.tensor.matmul(pscnt[:], onesc_t[:], zbd[:],
                                     start=True, stop=True)
                    cntinv = smpool.tile([1, 512], F32, tag="cntinv")
                    nc.vector.tensor_scalar(out=cntinv[:], in0=pscnt[:],
                                            scalar1=1.0, scalar2=None,
                                            op0=mybir.AluOpType.max)
                    nc.vector.reciprocal(out=cntinv[:], in_=cntinv[:])
                    psrep = ps3pool.tile([128, 512], F32, tag="psrep")
                    nc.tensor.matmul(psrep[:], ones1_t[:], cntinv[:],
                                     start=True, stop=True)
                    # note: psrep = cntinv^T replicated? see host mapping
                    cntrep = smpool.tile([128, 512], F32, tag="cntrep")
                    nc.vector.tensor_copy(cntrep[:], psrep[:])

                    # einsum2: out[o, s] accumulated over p
                    pse2 = ps2pool.tile([128, 512], F32, tag="pse2")
                    for p in range(P if "e2" not in SKIP else 1):
                        nc.tensor.matmul(
                            pse2[:],
                            ap_view(wp_t[:], p * C_OUT, [[1, C_OUT]]),
                            ap_view(wtt[:], p,
                                    [[480, 16], [120, 4], [15, 8]]),
                            start=(p == 0), stop=True)
                    # divide by count, add bias
                    e2sb = fpool.tile([128, 512], F32, tag="e2sb")
                    nc.vector.tensor_tensor(out=e2sb[:], in0=pse2[:],
                                            in1=cntrep[:],
                                            op=mybir.AluOpType.mult)
                    nc.vector.tensor_scalar(out=e2sb[:], in0=e2sb[:],
                                            scalar1=bias_t[:],
                                            scalar2=None,
                                            op0=mybir.AluOpType.add)
                    # transpose 4x128 cols and store
                    for t4 in range(4):
                        pstr = ps3pool.tile([128, 128], F32, tag="pstr")
                        nc.tensor.transpose(
                            pstr[:], e2sb[:, t4 * 128:(t4 + 1) * 128],
                            ident_t[:])
                        trsb = fpool.tile([128, 128], F32, tag="trsb")
                        nc.scalar.copy(trsb[:], pstr[:])
                        # e2 cols are n-linear: plain contiguous store
                        n0 = st * 512 + t4 * 128
                        nc.sync.dma_start(out_t[n0:n0 + 128, :], trsb[:])


def _make_runner(nc, n_cores):
    bass2jax.install_neuronx_cc_hook()
    from jax.sharding import Mesh, PartitionSpec
    from jax.experimental.shard_map import shard_map

    partition_name = nc.partition_id_tensor.name if nc.partition_id_tensor else None
    in_names, out_names, out_avals, zero_outs = [], [], [], []
    for alloc in nc.m.functions[0].allocations:
        if not isinstance(alloc, mybir.MemoryLocationSet):
            continue
        name = alloc.memorylocations[0].name
        if alloc.kind == "ExternalInput":
            if name != partition_name:
                in_names.append(name)
        elif alloc.kind == "ExternalOutput":
            shape = tuple(alloc.tensor_shape)
            dtype = mybir.dt.np(alloc.dtype)
            out_names.append(name)
            out_avals.append(jax.core.ShapedArray(shape, dtype))
            zero_outs.append(np.zeros(shape, dtype))
    n_params = len(in_names)
    n_outs = len(out_avals)
    all_in = in_names + out_names + ([partition_name] if partition_name else [])

    def _body(*args):
        operands = list(args)
        if partition_name is not None:
            operands.append(bass2jax.partition_id_tensor())
        outs = bass2jax._bass_exec_p.bind(
            *operands, out_avals=tuple(out_avals), in_names=tuple(all_in),
            out_names=tuple(out_names), lowering_input_output_aliases=(),
            sim_require_finite=False, sim_require_nnan=False, nc=nc)
        return tuple(outs)

    devices = jax.devices()[:n_cores]
    mesh = Mesh(np.asarray(devices), ("core",))
    in_specs = (PartitionSpec("core"),) * (n_params + n_outs)
    out_specs = (PartitionSpec("core"),) * n_outs
    jit_fn = jax.jit(
        shard_map(_body, mesh=mesh, in_specs=in_specs, out_specs=out_specs,
                  check_rep=False), keep_unused=True)

    def run(in_maps):
        per_core = [[np.asarray(m[n]) for n in in_names] for m in in_maps]
        args = [np.concatenate([per_core[c][i] for c in range(n_cores)], axis=0)
                for i in range(n_params)]
        args += [np.zeros((n_cores * z.shape[0], *z.shape[1:]), z.dtype)
                 for z in zero_outs]
        outs = [np.asarray(o) for o in jit_fn(*args)]
        return [{n: outs[i].reshape(n_cores, *out_avals[i].shape)[c]
                 for i, n in enumerate(out_names)}
                for c in range(n_cores)], jit_fn, args

    return run


_BUILT = {}


def _get_runner(kp):
    key = kp.tobytes()
    if key not in _BUILT:
        nc = build_bass(kp)
        _BUILT[key] = _make_runner(nc, N_CORES)
    return _BUILT[key]


def _host_prep(query_points, support_points, support_features,
               neighbor_indices, weights, bias, kernel_points):
    qp = np.asarray(query_points, np.float32)
    sp = np.asarray(support_points, np.float32)
    sf = np.asarray(support_features, np.float32)
    ni = np.asarray(neighbor_indices)
    ni = np.clip(ni, 0, M - 1).astype(np.int16)
    w = np.ascontiguousarray(np.asarray(weights, np.float32))
    bias = np.asarray(bias, np.float32).reshape(C_OUT, 1)

    mask120 = np.zeros((128, 120), np.float32)
    for q in range(8):
        mask120[q * 16:(q + 1) * 16, q * 15:(q + 1) * 15] = 1.0
    mask16 = np.zeros((128, 8), np.float32)
    for q in range(8):
        mask16[q * 16:(q + 1) * 16, q] = 1.0
    ident = np.eye(128, dtype=np.float32)
    ones1 = np.ones((1, 128), np.float32)
    kpv = np.asarray(kernel_points, np.float32)
    kpb = np.zeros((128, 48), np.float32)
    for p in range(P):
        for d in range(3):
            kpb[:, 3 * p + d] = -kpv[p, d]
    kpb[:, 45] = 1e-10
    kpb[:, 46] = -1.0 / SIGMA

    in_maps = []
    for c in range(N_CORES):
        b, half = divmod(c, 2)
        n0 = half * NQ_CORE
        idx = ni[b, n0:n0 + NQ_CORE, :].reshape(NK_CORE)
        # chunk order: idx j in chunk -> partition j%16 (k), col j//16;
        # stream order is already (query, k) = natural
        idx_l = idx.reshape(NK_CORE // 16, 16).T          # [16, NK/16]
        idx_l = np.tile(idx_l, (8, 1))                    # [128, NK/16]
        qrep = np.repeat(qp[b, n0:n0 + NQ_CORE, :], K, axis=0)  # [NK, 3]
        qrep = qrep.reshape(NK_CORE // 128, 128, 3).transpose(1, 0, 2)
        qrep = np.ascontiguousarray(qrep)
        in_maps.append({
            "sfeat": sf[b], "spts": sp[b], "qrep": qrep,
            "idx": np.ascontiguousarray(idx_l),
            "w": w, "bias": bias, "mask120": mask120, "mask16": mask16,
            "ident": ident, "ones1": ones1, "kpb": kpb,
            "onesc": np.ones((128, 1), np.float32),
        })
    return in_maps


# ===========================================================================
# Sparse path: kw = relu(1 - d/sigma) is ~99.99% zero for these inputs
# (support/query points uniform in [0,1]^3, sigma=0.03). Host finds a
# conservative SUPERSET of candidate (query, neighbor) pairs by integer
# cell binning (no float math decides values, only candidate pruning; any
# pair within reach of any kernel point is provably included). The device
# gathers those pairs' coords + features, computes exact kw and the two
# einsums for just those pairs, and scatter-adds into the bias-prefilled
# output. Falls back to the dense kernel when candidates exceed CAP.
# ===========================================================================
CAP = 5120          # static per-core candidate-pair capacity (40 blocks)
GRID = 128          # cells per axis for host binning
NBLK = CAP // 128
TRASH = NQ_CORE     # out_t row 8192 = trash for pad/unused slots


def build_sparse(reps=0, skip=()):
    sk = set(skip)
    nc = bass.Bass(dynamic_dma_scratch_size=32768, num_swdge_queues=4)

    ftab_in = nc.dram_tensor("ftab", [M, 128], F16, kind="ExternalInput")
    qsel_in = nc.dram_tensor("qsel", [128, (CAP // 128) * 4], F32,
                             kind="ExternalInput")
    wcat_in = nc.dram_tensor("wcat", [128, 2048], F16, kind="ExternalInput")
    kpcat_in = nc.dram_tensor("kpcat", [128, 48], F32, kind="ExternalInput")
    seg_in = nc.dram_tensor("seg", [128, CAP], F16, kind="ExternalInput")
    ident_in = nc.dram_tensor("ident16", [128, 128], F16, kind="ExternalInput")
    midx_in = nc.dram_tensor("midx", [128, CAP // 16], I16, kind="ExternalInput")
    nscidx_in = nc.dram_tensor("nscidx", [128, CAP // 16], I16, kind="ExternalInput")
    out_t = nc.dram_tensor("out", [NQ_CORE + 1, C_OUT], F32, kind="ExternalOutput")

    nc.gpsimd.load_library(library_config.mlp)

    with TileContext(nc) as tc:
        with tc.tile_pool(name="const", bufs=1) as cpool, \
             tc.tile_pool(name="gath", bufs=1) as gpool, \
             tc.tile_pool(name="work", bufs=2) as wpool, \
             tc.tile_pool(name="psf", bufs=2, space="PSUM") as psfpool, \
             tc.tile_pool(name="pst", bufs=2, space="PSUM") as pstpool, \
             tc.tile_pool(name="ps2", bufs=2, space="PSUM") as ps2pool:
            wcat_t = cpool.tile([128, 2048], F16, tag="wcat")
            nc.sync.dma_start(wcat_t[:], wcat_in[:])
            kpcat_t = cpool.tile([128, 48], F32, tag="kpcat")
            nc.sync.dma_start(kpcat_t[:], kpcat_in[:])
            seg_t = cpool.tile([128, CAP], F16, tag="seg")
            nc.sync.dma_start(seg_t[:], seg_in[:])
            ident_t = cpool.tile([128, 128], F16, tag="ident16")
            nc.sync.dma_start(ident_t[:], ident_in[:])
            midx_t = cpool.tile([128, CAP // 16], I16, tag="midx")
            nc.sync.dma_start(midx_t[:], midx_in[:])

            nscidx_t = cpool.tile([128, CAP // 16], I16, tag="nscidx")
            nc.sync.dma_start(nscidx_t[:], nscidx_in[:])
            nreg = nc.gpsimd.to_reg(CAP)
            greg = nc.gpsimd.to_reg(1024)
            done_sems = [nc.alloc_semaphore(f"scat_done{q}") for q in range(4)]
            swctr = [0]

            def swq():
                swctr[0] += 1
                return 0

            import contextlib
            loop_cm = tc.For_i(0, reps, 1) if reps else contextlib.nullcontext()
            with loop_cm:
                # gathers: fT (transpose mode), combined row (feats+coords),
                # query row
                GCH = 1024  # indices per dma_gather call
                NCH = CAP // GCH             # chunks
                BPC = GCH // 128             # blocks per chunk
                qsb = gpool.tile([128, NBLK, 4], F32, tag="qsb")
                nc.sync.dma_start(qsb[:], qsel_in[:].rearrange(
                    "a (b c) -> a b c", c=4))
                ssb_l, ftg_l, kwt_l = [], [], []
                for g in range(NCH):
                    ssb = gpool.tile([128, BPC, 64], F32, tag=f"ssb{g}")
                    if "ssb" in sk:
                        nc.vector.memset(ssb[:], 0.0)
                    else:
                        nc.gpsimd.dma_gather(
                            ssb[:], ftab_in[:].bitcast(F32),
                            midx_t[:, g * GCH // 16:(g + 1) * GCH // 16],
                            GCH, greg, 64, queue_num=swq())
                    ssb_l.append(ssb)
                    # fT via PE transpose, one 128-entry block at a time
                    ftg = gpool.tile([128, GCH], F16, tag=f"ftg{g}")
                    ssb16 = ssb[:].bitcast(F16)      # [128, BPC, 128]
                    for cc in range(BPC):
                        psumT = pstpool.tile([64, 128], F16, tag="pst")
                        nc.tensor.transpose(
                            psumT[:],
                            bass.AP(ssb16.tensor, ssb16.offset + cc * 128,
                                    [ssb16.ap[0], [1, 64]]),
                            ident_t[:])
                        nc.scalar.copy(
                            ap_part(ftg[:], 0, 64, cc * 128, [[1, 128]]),
                            psumT[:])
                    ftg_l.append(ftg)
                    # rel = s - q; kw = relu(1 - sqrt(d2)/sigma)
                    rel = gpool.tile([128, BPC, 3], F32, tag=f"rel{g}")
                    nc.vector.tensor_tensor(
                        out=rel[:],
                        in0=ap_view(ssb[:], 32, [[64, BPC], [1, 3]]),
                        in1=ap_view(qsb[:], g * BPC * 4, [[4, BPC], [1, 3]]),
                        op=mybir.AluOpType.subtract)
                    diff = gpool.tile([128, BPC * 45], F32, tag=f"diff{g}")
                    nc.vector.tensor_tensor(
                        out=ap_view(diff[:], 0, [[45, BPC], [3, P], [1, 3]]),
                        in0=ap_view(rel[:], 0, [[3, BPC], [0, P], [1, 3]]),
                        in1=ap_view(kpcat_t[:], 0, [[0, BPC], [3, P], [1, 3]]),
                        op=mybir.AluOpType.subtract)
                    nc.scalar.activation(diff[:], diff[:],
                                         mybir.ActivationFunctionType.Square,
                                         bias=0.0, scale=1.0)
                    kwt = gpool.tile([128, BPC, 16], F32, tag=f"kw{g}")
                    nc.vector.memset(kwt[:], 0.0)
                    nc.vector.tensor_reduce(
                        out=ap_view(kwt[:], 0, [[16, BPC], [1, P]]),
                        in_=ap_view(diff[:], 0, [[45, BPC], [3, P], [1, 3]]),
                        axis=mybir.AxisListType.X, op=mybir.AluOpType.add)
                    nc.scalar.activation(kwt[:], kwt[:],
                                         mybir.ActivationFunctionType.Sqrt,
                                         bias=kpcat_t[:, 45:46], scale=1.0)
                    nc.scalar.activation(kwt[:], kwt[:],
                                         mybir.ActivationFunctionType.Relu,
                                         bias=1.0, scale=kpcat_t[:, 46:47])
                    kwt_l.append(kwt)

                scat = gpool.tile([128, NBLK, C_OUT], F32, tag="scat")
                lp = nc.allow_low_precision(
                    reason="f16 weighted intermediates; validated vs "
                           "reference at 3e-4 rel err")
                lp.__enter__()
                for c in range(NBLK):
                    wtdm_a = wpool.tile([128, 1024], F16, tag="wtdma")
                    wtdm_b = wpool.tile([128, 1024], F16, tag="wtdmb")
                    wtdm_hw = (wtdm_a, wtdm_b)
                    for hw in range(2):  # p 0:8 | p 8:16 (slot 15 zero-W)
                        psumF = psfpool.tile([128, 1024], F32, tag="psf")
                        for k in range(2):
                            nc.tensor.matmul(
                                psumF[:, k * 512:(k + 1) * 512],
                                ap_part(ftg_l[c // BPC][:], 0, C_IN,
                                        (c % BPC) * 128, [[1, 128]]),
                                ap_part(wcat_t[:], 0, C_IN,
                                        hw * 1024 + k * 512, [[1, 512]]),
                                start=True, stop=True)
                        fw16 = wpool.tile([128, 1024], F16, tag="fw16")
                        nc.scalar.copy(fw16[:], psumF[:])
                        nc.vector.tensor_tensor(
                            out=ap_view(wtdm_hw[hw][:], 0,
                                        [[C_OUT, 8], [1, C_OUT]]),
                            in0=ap_view(fw16[:], 0, [[C_OUT, 8], [1, C_OUT]]),
                            in1=ap_view(kwt_l[c // BPC][:],
                                        (c % BPC) * 16 + hw * 8,
                                        [[1, 8], [0, C_OUT]]),
                            op=mybir.AluOpType.mult)
                    tr1 = wpool.tile([128, 1024], F16, tag="tr1")
                    nc.vector.tensor_tensor(
                        out=tr1[:], in0=wtdm_a[:], in1=wtdm_b[:],
                        op=mybir.AluOpType.add)
                    tr2 = wpool.tile([128, 512], F16, tag="tr2")
                    nc.vector.tensor_tensor(
                        out=tr2[:], in0=tr1[:, 0:512], in1=tr1[:, 512:1024],
                        op=mybir.AluOpType.add)
                    tr3 = wpool.tile([128, 256], F16, tag="tr3")
                    nc.vector.tensor_tensor(
                        out=tr3[:], in0=tr2[:, 0:256], in1=tr2[:, 256:512],
                        op=mybir.AluOpType.add)
                    ct = wpool.tile([128, C_OUT], F16, tag="ct")
                    nc.vector.tensor_tensor(
                        out=ct[:], in0=tr3[:, 0:128], in1=tr3[:, 128:256],
                        op=mybir.AluOpType.add)
                    psum2 = ps2pool.tile([128, C_OUT], F32, tag="ps2")
                    nc.tensor.matmul(psum2[:], seg_t[:, c * 128:(c + 1) * 128],
                                     ct[:], start=True, stop=True)
                    nc.scalar.copy(
                        ap_view(scat[:], c * C_OUT, [[1, C_OUT]]), psum2[:])

                lp.__exit__(None, None, None)
                if "scatter" not in sk:
                    qcnt = [0, 0, 0, 0]
                    for g in range(CAP // GCH):
                        q = 0
                        nc.gpsimd.dma_scatter_add(
                            out_t[:],
                            ap_view(scat[:], g * (GCH // 128) * C_OUT,
                                    [[C_OUT, GCH // 128], [1, C_OUT]]),
                            nscidx_t[:, g * GCH // 16:(g + 1) * GCH // 16],
                            GCH, greg, C_OUT,
                            queue_num=q).then_inc(done_sems[q], 16)
                        qcnt[q] += 16
                    for q in range(4):
                        if qcnt[q]:
                            nc.gpsimd.wait_ge(done_sems[q], qcnt[q])
                else:
                    nc.sync.dma_start(out_t[0:128, :],
                                      ap_view(scat[:], 0, [[1, C_OUT]]))
    return nc


def _make_runner_sparse(nc, n_cores):
    bass2jax.install_neuronx_cc_hook()
    from jax.sharding import Mesh, PartitionSpec
    from jax.experimental.shard_map import shard_map

    partition_name = nc.partition_id_tensor.name if nc.partition_id_tensor else None
    in_names, out_names, out_avals = [], [], []
    for alloc in nc.m.functions[0].allocations:
        if not isinstance(alloc, mybir.MemoryLocationSet):
            continue
        name = alloc.memorylocations[0].name
        if alloc.kind == "ExternalInput":
            if name != partition_name:
                in_names.append(name)
        elif alloc.kind == "ExternalOutput":
            shape = tuple(alloc.tensor_shape)
            dtype = mybir.dt.np(alloc.dtype)
            out_names.append(name)
            out_avals.append(jax.core.ShapedArray(shape, dtype))
    n_params = len(in_names)
    n_outs = len(out_avals)
    all_in = in_names + out_names + ([partition_name] if partition_name else [])

    def _body(*args):
        operands = list(args)
        if partition_name is not None:
            operands.append(bass2jax.partition_id_tensor())
        outs = bass2jax._bass_exec_p.bind(
            *operands, out_avals=tuple(out_avals), in_names=tuple(all_in),
            out_names=tuple(out_names), lowering_input_output_aliases=(),
            sim_require_finite=False, sim_require_nnan=False, nc=nc)
        return tuple(outs)

    devices = jax.devices()[:n_cores]
    mesh = Mesh(np.asarray(devices), ("core",))
    in_specs = (PartitionSpec("core"),) * (n_params + n_outs)
    out_specs = (PartitionSpec("core"),) * n_outs
    donate = tuple(range(n_params, n_params + n_outs))
    jit_fn = jax.jit(
        shard_map(_body, mesh=mesh, in_specs=in_specs, out_specs=out_specs,
                  check_rep=False), donate_argnums=donate, keep_unused=True)

    def run(in_maps, out_prefills):
        per_core = [[np.asarray(m[n]) for n in in_names] for m in in_maps]
        args = [np.concatenate([per_core[c][i] for c in range(n_cores)], axis=0)
                for i in range(n_params)]
        args += [np.concatenate([np.asarray(p[n]) for p in out_prefills], axis=0)
                 for n in out_names]
        outs = [np.asarray(o) for o in jit_fn(*args)]
        return [{n: outs[i].reshape(n_cores, *out_avals[i].shape)[c]
                 for i, n in enumerate(out_names)}
                for c in range(n_cores)], jit_fn, args

    return run


def _get_runner_sparse():
    if "sparse" not in _BUILT:
        nc = build_sparse()
        _BUILT["sparse"] = _make_runner_sparse(nc, N_CORES)
    return _BUILT["sparse"]


def _wrap16(vals, pad_val, dtype=np.int16):
    """List -> [128, CAP//16] wrapped (entry j at [j%16, j//16]), replicated
    across the 8 gpsimd cores."""
    buf = np.full(CAP, pad_val, dtype)
    buf[:len(vals)] = vals
    w = buf.reshape(CAP // 16, 16).T          # [16, CAP//16]
    return np.ascontiguousarray(np.tile(w, (8, 1)))


def _host_prep_sparse(qp, sp, sf, ni, w, bias_v, kpv):
    """Returns (in_maps, out_prefills) or None if candidates exceed CAP."""
    # conservative candidate radius: a hit needs |s - q| < sigma + max|kp|
    rmax = SIGMA + float(np.sqrt((kpv * kpv).sum(axis=1)).max())
    t_cell = (rmax * GRID + math.sqrt(3.0)) ** 2
    scell = np.clip((sp * GRID).astype(np.int32), 0, GRID - 1)
    qcell = np.clip((qp * GRID).astype(np.int32), 0, GRID - 1)

    wcat = np.zeros((128, 2048), np.float16)
    wcat[:C_IN, :P * C_OUT] = (np.transpose(w, (1, 0, 2)) / 16.0
                               ).reshape(C_IN, -1)
    wcat[C_IN:] = wcat[:C_IN]
    kpcat = np.zeros((128, 48), np.float32)
    kpcat[:, :45] = kpv.reshape(1, 45)
    kpcat[:, 45] = 1e-10
    kpcat[:, 46] = -1.0 / SIGMA

    in_maps, out_prefills = [], []
    for c in range(N_CORES):
        b, half = divmod(c, 2)
        n0 = half * NQ_CORE
        nib = ni[b, n0:n0 + NQ_CORE]
        dc = scell[b][nib] - qcell[b, n0:n0 + NQ_CORE, None, :]
        d2 = (dc.astype(np.int64) ** 2).sum(axis=2)
        nn, kk = np.nonzero(d2 <= t_cell)      # sorted by n (row-major)
        mm = nib[nn, kk]

        # pack into 128-entry blocks; no query group spans a block boundary
        m_list = np.zeros(CAP, np.int16)
        n_list = np.zeros(CAP, np.int16)
        seg = np.zeros((128, CAP), np.float16)
        sc_idx = np.full(CAP, TRASH, np.int16)
        uniq, counts = np.unique(nn, return_counts=True)
        t = 0            # global entry cursor
        gi = 0           # group cursor
        ok = True
        off = 0          # start of each group's pairs in nn/kk
        for g in range(len(uniq)):
            cnt = counts[g]
            blk, pos = divmod(t, 128)
            if pos + cnt > 128:                # pad to next block
                t = (blk + 1) * 128
                blk, pos = blk + 1, 0
            if t + cnt > CAP:
                ok = False
                break
            d = 127                            # d-slot for this group
            # d slots allocated in order of first use within the block
            # (track per-block next free slot)
            m_list[t:t + cnt] = mm[off:off + cnt]
            n_list[t:t + cnt] = uniq[g]
            t += cnt
            off += cnt
        if not ok:
            return None
        # second pass: assign d-slots and seg/sc_idx now that layout is fixed
        seg[:] = 0
        sc_idx[:] = TRASH
        blk_next = np.zeros(NBLK, np.int32)
        t = 0
        off = 0
        for g in range(len(uniq)):
            cnt = counts[g]
            blk, pos = divmod(t, 128)
            if pos + cnt > 128:
                t = (blk + 1) * 128
                blk, pos = blk + 1, 0
            d = blk_next[blk]
            blk_next[blk] += 1
            seg[pos:pos + cnt, blk * 128 + d] = 1.0
            sc_idx[blk * 128 + d] = uniq[g]
            t += cnt
            off += cnt
        # pad entries (between groups / tail): m=0, n=0 gathers; their seg
        # column stays 0 -> contribute nothing; unused d-slots scatter to
        # TRASH row.

        ftab = np.zeros((M, 128), np.float16)
        ftab[:, :C_IN] = sf[b]
        ftab.view(np.float32)[:, 32:35] = sp[b]
        qsel = np.zeros((CAP, 4), np.float32)
        qsel[:, :3] = qp[b, n0 + n_list.astype(np.int64)]
        qsel = np.ascontiguousarray(
            qsel.reshape(NBLK, 128, 4).transpose(1, 0, 2)).reshape(128, -1)
        in_maps.append({
            "ftab": ftab, "qsel": qsel, "wcat": wcat, "kpcat": kpcat,
            "seg": seg, "ident16": np.eye(128, dtype=np.float16),
            "midx": _wrap16(m_list, 0),
            "nscidx": _wrap16(sc_idx, TRASH),
        })
        out_prefills.append({
            "out": np.tile(bias_v.reshape(1, C_OUT),
                           (NQ_CORE + 1, 1)).astype(np.float32)})
    return in_maps, out_prefills


def _kernel_dense(qp_raw, sp_raw, sf_raw, ni_raw, w_raw, bias_raw, kp_raw):
    kp = np.asarray(kp_raw, np.float32)
    run = _get_runner(kp)
    in_maps = _host_prep(qp_raw, sp_raw, sf_raw, ni_raw, w_raw, bias_raw,
                         kp_raw)
    results, _, _ = run(in_maps)
    out = np.zeros((B, N, C_OUT), np.float32)
    for c in range(N_CORES):
        b, half = divmod(c, 2)
        n0 = half * NQ_CORE
        out[b, n0:n0 + NQ_CORE, :] = results[c]["out"]
    return out


def kernel(query_points, support_points, support_features, neighbor_indices,
           weights, bias, kernel_points):
    qp = np.asarray(query_points, np.float32)
    sp = np.asarray(support_points, np.float32)
    sf = np.asarray(support_features, np.float32)
    ni = np.clip(np.asarray(neighbor_indices), 0, M - 1).astype(np.int32)
    w = np.asarray(weights, np.float32)
    bias_v = np.asarray(bias, np.float32)
    kpv = np.asarray(kernel_points, np.float32)

    prep = _host_prep_sparse(qp, sp, sf, ni, w, bias_v, kpv)
    if prep is None:
        return _kernel_dense(query_points, support_points, support_features,
                             neighbor_indices, weights, bias, kernel_points)
    in_maps, out_prefills = prep
    run = _get_runner_sparse()
    results, _, _ = run(in_maps, out_prefills)
    out = np.zeros((B, N, C_OUT), np.float32)
    for c in range(N_CORES):
        b, half = divmod(c, 2)
        n0 = half * NQ_CORE
        out[b, n0:n0 + NQ_CORE, :] = results[c]["out"][:NQ_CORE]

    # exact neighbor-count correction (reference divides by the number of
    # neighbors with nonzero features, clipped to >= 1; the device divides
    # by K=16). For randn features every row is nonzero, so cnt == 16 and
    # this is a no-op; handle degenerate inputs on host for full fidelity.
    row_nz = np.abs(sf).sum(axis=2) > 0          # [B, M]
    if not row_nz.all():
        z = row_nz.astype(np.float32)
        cnt = np.clip(
            z[np.arange(B)[:, None, None], ni].sum(axis=2), 1.0, None)
        out = (out - bias_v) * (16.0 / cnt)[..., None] + bias_v
    return out



# revision 36
# speedup vs baseline: 1.0085x; 1.0085x over previous
"""KPConv (nn_KPConvFPN) Trainium2 Bass kernel — sparse candidate-pair design.

kw = relu(1 - |s[m] - q[n] - kp_p|/sigma) is ~97.6% zero for these inputs
(points uniform in [0,1]^3, sigma + max|kp| = 0.0825). The host finds a
conservative SUPERSET of candidate (query, neighbor) pairs by integer cell
binning (GRID=128; any pair within reach of any kernel point is provably
included; no float math decides output values on the host). Per core
(batch b=c//2, query half c%2):

Device pipeline (CAP=5120 candidate pairs, 40 blocks of 128):
  1. Per 1024-pair chunk: SWDGE dma_gather of combined 256B rows
     [64 f16 feats | s-coords f32] from ftab; PE-transpose feats -> fT;
     DVE/ACT compute kw[t, p] for all 15 kernel points.
     (query coords arrive pre-gathered from host as qsel, like the dense
     kernel's qrep.)
  2. Per 128-pair block: 4 PE matmuls fW = fT @ [W_0|..|W_15]/16 (f16,
     2048 psum cols); DVE multiply by kw broadcast over C_out; binary-tree
     add over the 16 p-slots -> ct[t, 128].
  3. Segment matmul psum2[d, o] = seg[t, d]^T @ ct (host-built 0/1 seg
     matrix; groups pairs of the same query; pads/unused -> trash slot),
     so every output row is scattered EXACTLY once (dma_scatter_add loses
     updates on duplicate rows -- measured).
  4. dma_scatter_add rows into the bias-prefilled donated output buffer
     (row 8192 = trash row for pad slots).

Falls back to the dense kernel (build_bass below) when candidates exceed
CAP. The reference divides by the count of neighbors with nonzero
features; for randn features that is always K=16 (folded into W/16); the
degenerate case is corrected exactly on the host.
"""
import json
import math
import os

SKIP = set()

import numpy as np
import jax

import concourse.bass as bass
import concourse.mybir as mybir
from concourse.tile import TileContext
from concourse import library_config
from concourse import bass2jax

F32 = mybir.dt.float32
F16 = mybir.dt.float16
I16 = mybir.dt.int16

B, N, M, K = 4, 16384, 16384, 16
C_IN, C_OUT, P = 64, 128, 15
SIGMA = 0.03
N_CORES = 8
NQ_CORE = N // 2            # 8192 queries per core
NK_CORE = NQ_CORE * K       # 131072 gathered rows per core
ST_Q = 512                  # queries per supertile
N_ST = NQ_CORE // ST_Q      # 16
KW_ST = 2                   # supertiles per kw group
G_ST = ST_Q * K // 128      # 64 g-cols per supertile
ROW16 = 128                 # fp16 units per table row (256B)

# ---------------------------------------------------------------------------
# walrus workaround: this nix walrus build supports ONE sync-wait per
# instruction; split extra waits onto NoOps inserted before the offender
# (same-engine program order preserves semantics). Also run
# codegen_inst_isa_subclasses (Bacc does; raw Bass doesn't) so extended
# instructions get their ISA bytes.
_orig_to_json_bytes = bass.Bass.to_json_bytes


def _fix_block(bb, ctr):
    insts = bb.get("instructions")
    if not isinstance(insts, list):
        return
    new = []
    for inst in insts:
        si = inst.get("sync_info")
        ow = si.get("on_wait") if isinstance(si, dict) else None
        if ow and len(ow) > 1:
            for w in ow[:-1]:
                ctr[0] += 1
                nop = {"engine": inst["engine"], "ins": [], "outs": [],
                       "name": f"I-wsplit-{ctr[0]}", "opcode": "NoOp",
                       "sync_info": {"on_update": [], "on_wait": [w]},
                       "text_hint": "wsplit"}
                if "debug" in inst:
                    nop["debug"] = inst["debug"]
                new.append(nop)
            si["on_wait"] = [ow[-1]]
        new.append(inst)
    bb["instructions"] = new


def _walk(o, ctr):
    if isinstance(o, dict):
        if isinstance(o.get("instructions"), list):
            _fix_block(o, ctr)
        for v in o.values():
            _walk(v, ctr)
    elif isinstance(o, list):
        for v in o:
            _walk(v, ctr)


def _to_json_bytes_split(self):
    mybir.codegen_inst_isa_subclasses(self)
    raw = _orig_to_json_bytes(self)
    d = json.loads(raw)
    ctr = [0]
    _walk(d, ctr)
    return json.dumps(d).encode()


bass.Bass.to_json_bytes = _to_json_bytes_split


def ap_view(t_ap, extra_offset, dims):
    """AP over tile t_ap with explicit free dims [[step, count], ...]
    (steps in elements); partition dim is taken from the tile."""
    return bass.AP(t_ap.tensor, t_ap.offset + extra_offset,
                   [t_ap.ap[0]] + list(dims))


def ap_part(t_ap, pstart, pcount, extra_offset, dims):
    pstep = t_ap.ap[0][0]
    return bass.AP(t_ap.tensor, t_ap.offset + pstart * pstep + extra_offset,
                   [[pstep, pcount]] + list(dims))


def build_bass(kp, reps=0, skip=()):
    global SKIP
    SKIP = set(skip)
    """kp: (15, 3) float32 numpy kernel points (runtime values baked)."""
    kpsq = (kp * kp).sum(axis=1)  # |kp_p|^2
    nc = bass.Bass(dynamic_dma_scratch_size=32768, num_swdge_queues=4)

    feats_in = nc.dram_tensor("sfeat", [M, C_IN], F32, kind="ExternalInput")
    pts_in = nc.dram_tensor("spts", [M, 3], F32, kind="ExternalInput")
    qrep_in = nc.dram_tensor("qrep", [128, NK_CORE // 128, 3], F32,
                             kind="ExternalInput")
    idx_in = nc.dram_tensor("idx", [128, NK_CORE // 16], I16,
                            kind="ExternalInput")
    w_in = nc.dram_tensor("w", [P, C_IN, C_OUT], F32, kind="ExternalInput")
    bias_in = nc.dram_tensor("bias", [C_OUT, 1], F32, kind="ExternalInput")
    mask120_in = nc.dram_tensor("mask120", [128, 120], F32, kind="ExternalInput")
    mask16_in = nc.dram_tensor("mask16", [128, 8], F32, kind="ExternalInput")
    ident_in = nc.dram_tensor("ident", [128, 128], F32, kind="ExternalInput")
    ones1_in = nc.dram_tensor("ones1", [1, 128], F32, kind="ExternalInput")
    kpb_in = nc.dram_tensor("kpb", [128, 48], F32, kind="ExternalInput")
    onesc_in = nc.dram_tensor("onesc", [128, 1], F32, kind="ExternalInput")
    out_t = nc.dram_tensor("out", [NQ_CORE, C_OUT], F32, kind="ExternalOutput")
    table = nc.dram_tensor("table", [M, ROW16], F16, kind="Internal")

    # library load as raw preamble (before Tile scheduling) so it is
    # guaranteed to precede every dma_gather on the Pool engine.
    nc.gpsimd.load_library(library_config.mlp)

    with TileContext(nc) as tc:
        with tc.tile_pool(name="const", bufs=1) as cpool, \
             tc.tile_pool(name="build", bufs=1) as bpool, \
             tc.tile_pool(name="gath", bufs=2) as gpool, \
             tc.tile_pool(name="kwp", bufs=2) as kwpool, \
             tc.tile_pool(name="kbd", bufs=1) as kbpool, \
             tc.tile_pool(name="wt", bufs=1) as wtpool, \
             tc.tile_pool(name="sm", bufs=3) as smpool, \
             tc.tile_pool(name="fin", bufs=2) as fpool, \
             tc.tile_pool(name="ps1", bufs=2, space="PSUM") as ps1pool, \
             tc.tile_pool(name="ps2", bufs=2, space="PSUM") as ps2pool, \
             tc.tile_pool(name="ps3", bufs=1, space="PSUM") as ps3pool:

            # ---- constants ----
            wp_t = cpool.tile([C_IN, P * C_OUT], F32, tag="wp")
            nc.sync.dma_start(
                wp_t[:].rearrange("c (p o) -> c p o", p=P),
                w_in[:].rearrange("p c o -> c p o"))
            bias_t = cpool.tile([C_OUT, 1], F32, tag="bias")
            nc.sync.dma_start(bias_t[:], bias_in[:])
            mask120_t = cpool.tile([128, 120], F32, tag="m120")
            nc.sync.dma_start(mask120_t[:], mask120_in[:])
            mask16_t = cpool.tile([128, 8], F32, tag="m16")
            nc.sync.dma_start(mask16_t[:], mask16_in[:])
            ident_t = cpool.tile([128, 128], F32, tag="ident")
            nc.sync.dma_start(ident_t[:], ident_in[:])
            ones1_t = cpool.tile([1, 128], F32, tag="ones1")
            nc.sync.dma_start(ones1_t[:], ones1_in[:])
            kpb_t = cpool.tile([128, 48], F32, tag="kpb")
            nc.sync.dma_start(kpb_t[:], kpb_in[:])
            onesc_t = cpool.tile([128, 1], F32, tag="onesc")
            nc.sync.dma_start(onesc_t[:], onesc_in[:])
            nidx_reg = nc.gpsimd.to_reg(1024)

            # ---- 1. combined table build (8 chunks x 2048 rows) ----
            import contextlib
            loop_cm = tc.For_i(0, reps, 1) if reps else contextlib.nullcontext()
            with loop_cm:
                _table_build(nc, tc, bpool, feats_in, pts_in, table)
                _main_pipeline(nc, tc, gpool, kwpool, kbpool, wtpool, smpool,
                               fpool, ps1pool, ps2pool, ps3pool, kp,
                               qrep_in, idx_in, out_t, table, wp_t, bias_t,
                               mask120_t, mask16_t, ident_t, ones1_t, kpb_t,
                               onesc_t, nidx_reg)
    return nc


def _table_build(nc, tc, bpool, feats_in, pts_in, table):
            for ch in range(8):
                m0 = ch * 2048
                fsb = bpool.tile([128, 16, C_IN], F32, tag="fsb")
                nc.sync.dma_start(
                    fsb[:],
                    feats_in[m0:m0 + 2048, :].rearrange(
                        "(a p) c -> p a c", p=128))
                psb = bpool.tile([128, 16, 3], F32, tag="psb")
                nc.sync.dma_start(
                    psb[:],
                    pts_in[m0:m0 + 2048, :].rearrange(
                        "(a p) c -> p a c", p=128))
                st16 = bpool.tile([128, 16, ROW16], F16, tag="st16")
                nc.vector.tensor_copy(st16[:, :, 0:C_IN], fsb[:])
                stv32 = st16[:].bitcast(F32)  # [128, 16, 64] f32 view
                # aux: sx sy sz at f32-cols 32..34
                nc.vector.tensor_copy(
                    bass.AP(stv32.tensor, stv32.offset + 32,
                            [stv32.ap[0], [64, 16], [1, 3]]),
                    psb[:])
                # |s|^2 at f32-col 35
                psq = bpool.tile([128, 16, 3], F32, tag="psq")
                nc.vector.tensor_tensor(out=psq[:], in0=psb[:], in1=psb[:],
                                        op=mybir.AluOpType.mult)
                nc.vector.tensor_reduce(
                    out=bass.AP(stv32.tensor, stv32.offset + 35,
                                [stv32.ap[0], [64, 16], [1, 1]]),
                    in_=psq[:], axis=mybir.AxisListType.X,
                    op=mybir.AluOpType.add)
                # z at f32-col 36: (sum_c |f|) > 0
                zred = bpool.tile([128, 16, 1], F32, tag="zred")
                nc.vector.tensor_reduce(out=zred[:], in_=fsb[:],
                                        axis=mybir.AxisListType.X,
                                        op=mybir.AluOpType.add,
                                        apply_absolute_value=True)
                nc.vector.tensor_scalar(
                    out=bass.AP(stv32.tensor, stv32.offset + 36,
                                [stv32.ap[0], [64, 16], [1, 1]]),
                    in0=zred[:], scalar1=0.0, scalar2=None,
                    op0=mybir.AluOpType.is_gt)
                nc.sync.dma_start(
                    table[m0:m0 + 2048, :].rearrange("(a p) c -> p a c",
                                                     p=128),
                    st16[:])


def _main_pipeline(nc, tc, gpool, kwpool, kbpool, wtpool, smpool, fpool,
                   ps1pool, ps2pool, ps3pool, kp, qrep_in, idx_in, out_t,
                   table, wp_t, bias_t, mask120_t, mask16_t, ident_t,
                   ones1_t, kpb_t, onesc_t, nidx_reg):
            for kg in range(N_ST // KW_ST):  # kw group of 2 supertiles
                GQ = KW_ST * ST_Q            # 1024 queries
                GG = KW_ST * G_ST            # 128 g-cols
                gt = gpool.tile([128, GG, ROW16], F16, tag="gath")
                gt32 = gt[:].bitcast(F32)  # [128, GG, 64] f32 view
                # gathers: 16 chunks of 1024 idx
                if "gather" in SKIP:
                    nc.vector.memset(gt[:], 0.0)
                for g in range(GG // 8):
                    if "gather" in SKIP:
                        break
                    idxsl = smpool.tile([128, 64], I16, tag="idxsl")
                    nc.sync.dma_start(
                        idxsl[:],
                        idx_in[:, (kg * 16 + g) * 64:(kg * 16 + g) * 64 + 64])
                    nc.gpsimd.dma_gather(
                        gt[:, g * 8:(g + 1) * 8, :], table[:], idxsl[:],
                        1024, nidx_reg, ROW16, queue_num=g % 4)
                # qrep slice
                qr = smpool.tile([128, GG, 3], F32, tag="qr")
                nc.sync.dma_start(qr[:], qrep_in[:, kg * GG:(kg + 1) * GG, :])
                # rel = s - q
                rel = smpool.tile([128, GG, 3], F32, tag="rel")
                nc.vector.tensor_tensor(
                    out=rel[:],
                    in0=ap_view(gt32, 32, [[64, GG], [1, 3]]),
                    in1=qr[:], op=mybir.AluOpType.subtract)
                # d2[p] = sum_dim (rel_dim - kp[p,dim])^2  (ACT squares, DVE adds)
                kwt = kwpool.tile([128, GG, P], F32, tag="kw")
                sq0 = smpool.tile([128, GG], F32, tag="sq0")
                sq1 = smpool.tile([128, GG], F32, tag="sq1")
                if "kw" in SKIP:
                    nc.vector.memset(kwt[:], 0.0)
                for p in range(P if "kw" not in SKIP else 0):
                    d2dst = ap_view(kwt[:], p, [[P, GG], [1, 1]])
                    nc.scalar.activation(
                        sq0[:], ap_view(rel[:], 0, [[3, GG], [1, 1]]),
                        mybir.ActivationFunctionType.Square,
                        bias=kpb_t[:, 3 * p:3 * p + 1], scale=1.0)
                    nc.scalar.activation(
                        sq1[:], ap_view(rel[:], 1, [[3, GG], [1, 1]]),
                        mybir.ActivationFunctionType.Square,
                        bias=kpb_t[:, 3 * p + 1:3 * p + 2], scale=1.0)
                    nc.vector.tensor_tensor(out=sq0[:], in0=sq0[:],
                                            in1=sq1[:],
                                            op=mybir.AluOpType.add)
                    nc.scalar.activation(
                        sq1[:], ap_view(rel[:], 2, [[3, GG], [1, 1]]),
                        mybir.ActivationFunctionType.Square,
                        bias=kpb_t[:, 3 * p + 2:3 * p + 3], scale=1.0)
                    nc.vector.tensor_tensor(out=d2dst, in0=sq0[:],
                                            in1=sq1[:],
                                            op=mybir.AluOpType.add)
                # kw = relu(1 - sqrt(d2 + 1e-10)/sigma), in place
                if "kw" in SKIP:
                    pass
                else:
                    nc.scalar.activation(kwt[:], kwt[:],
                                     mybir.ActivationFunctionType.Sqrt,
                                     bias=kpb_t[:, 45:46], scale=1.0)
                if "kw" not in SKIP:
                    nc.scalar.activation(kwt[:], kwt[:],
                                     mybir.ActivationFunctionType.Relu,
                                     bias=1.0, scale=kpb_t[:, 46:47])

                for sti in range(KW_ST):
                    st = kg * KW_ST + sti
                    # kwbd (2 half-ST TT ops): [128, (bl32, q8, p15)] fp16
                    kbd = kbpool.tile([128, 3840], F16, tag="kbd")
                    kbd2 = kbpool.tile([128, 3840], F16, tag="kbd2")
                    if "kwbd" in SKIP:
                        nc.vector.memset(kbd[:], 0.0)
                        nc.vector.memset(kbd2[:], 0.0)
                    for hf, kb in ((0, kbd), (1, kbd2)) if "kwbd" not in SKIP else ():
                        bl0 = sti * G_ST + hf * 32
                        nc.vector.tensor_tensor(
                            out=ap_view(kb[:], 0,
                                        [[120, 32], [15, 8], [1, 15]]),
                            in0=ap_view(kwt[:], bl0 * P,
                                        [[P, 32], [0, 8], [1, P]]),
                            in1=ap_view(mask120_t[:], 0,
                                        [[0, 32], [15, 8], [1, 15]]),
                            op=mybir.AluOpType.mult)
                    # einsum1: 64 blocks
                    wtt = wtpool.tile([64, 7680], F32, tag="wt")
                    if "e1" in SKIP:
                        nc.vector.memset(wtt[:], 0.0)
                    for bg in range(16 if "e1" not in SKIP else 0):  # bank groups of 4 blocks (32 q)
                        pse1 = ps1pool.tile([64, 480], F32, tag="pse1")
                        for j in range(4):
                            bl = bg * 4 + j          # block in supertile
                            blg = sti * G_ST + bl    # g-col in group tile
                            kb = kbd if bl < 32 else kbd2
                            kbl = bl % 32
                            nc.tensor.matmul(
                                pse1[:, j * 120:(j + 1) * 120],
                                ap_view(gt[:], blg * ROW16, [[1, C_IN]]),
                                ap_view(kb[:], kbl * 120, [[1, 120]]),
                                start=True, stop=True)
                        # evict (split DVE/ACT)
                        nc.vector.tensor_copy(
                            wtt[:, bg * 480:bg * 480 + 240],
                            pse1[:, 0:240])
                        nc.scalar.copy(
                            wtt[:, bg * 480 + 240:bg * 480 + 480],
                            pse1[:, 240:480])
                    # count row: zbd = z * mask16 -> ones-row matmul
                    zbd = smpool.tile([128, 512], F32, tag="zbd")
                    nc.vector.tensor_tensor(
                        out=zbd[:].rearrange("a (g j q) -> a g j q",
                                             g=16, j=4),
                        in0=ap_view(gt32, (sti * G_ST) * 64 + 36,
                                    [[256, 16], [64, 4], [0, 8]]),
                        in1=ap_view(mask16_t[:], 0,
                                    [[0, 16], [0, 4], [1, 8]]),
                        op=mybir.AluOpType.mult)
                    pscnt = ps3pool.tile([1, 512], F32, tag="pscnt")
                    nc.tensor.matmul(pscnt[:], onesc_t[:], zbd[:],
                                     start=True, stop=True)
                    cntinv = smpool.tile([1, 512], F32, tag="cntinv")
                    nc.vector.tensor_scalar(out=cntinv[:], in0=pscnt[:],
                                            scalar1=1.0, scalar2=None,
                                            op0=mybir.AluOpType.max)
                    nc.vector.reciprocal(out=cntinv[:], in_=cntinv[:])
                    psrep = ps3pool.tile([128, 512], F32, tag="psrep")
                    nc.tensor.matmul(psrep[:], ones1_t[:], cntinv[:],
                                     start=True, stop=True)
                    # note: psrep = cntinv^T replicated? see host mapping
                    cntrep = smpool.tile([128, 512], F32, tag="cntrep")
                    nc.vector.tensor_copy(cntrep[:], psrep[:])

                    # einsum2: out[o, s] accumulated over p
                    pse2 = ps2pool.tile([128, 512], F32, tag="pse2")
                    for p in range(P if "e2" not in SKIP else 1):
                        nc.tensor.matmul(
                            pse2[:],
                            ap_view(wp_t[:], p * C_OUT, [[1, C_OUT]]),
                            ap_view(wtt[:], p,
                                    [[480, 16], [120, 4], [15, 8]]),
                            start=(p == 0), stop=True)
                    # divide by count, add bias
                    e2sb = fpool.tile([128, 512], F32, tag="e2sb")
                    nc.vector.tensor_tensor(out=e2sb[:], in0=pse2[:],
                                            in1=cntrep[:],
                                            op=mybir.AluOpType.mult)
                    nc.vector.tensor_scalar(out=e2sb[:], in0=e2sb[:],
                                            scalar1=bias_t[:],
                                            scalar2=None,
                                            op0=mybir.AluOpType.add)
                    # transpose 4x128 cols and store
                    for t4 in range(4):
                        pstr = ps3pool.tile([128, 128], F32, tag="pstr")
                        nc.tensor.transpose(
                            pstr[:], e2sb[:, t4 * 128:(t4 + 1) * 128],
                            ident_t[:])
                        trsb = fpool.tile([128, 128], F32, tag="trsb")
                        nc.scalar.copy(trsb[:], pstr[:])
                        # e2 cols are n-linear: plain contiguous store
                        n0 = st * 512 + t4 * 128
                        nc.sync.dma_start(out_t[n0:n0 + 128, :], trsb[:])


def _make_runner(nc, n_cores):
    bass2jax.install_neuronx_cc_hook()
    from jax.sharding import Mesh, PartitionSpec
    from jax.experimental.shard_map import shard_map

    partition_name = nc.partition_id_tensor.name if nc.partition_id_tensor else None
    in_names, out_names, out_avals, zero_outs = [], [], [], []
    for alloc in nc.m.functions[0].allocations:
        if not isinstance(alloc, mybir.MemoryLocationSet):
            continue
        name = alloc.memorylocations[0].name
        if alloc.kind == "ExternalInput":
            if name != partition_name:
                in_names.append(name)
        elif alloc.kind == "ExternalOutput":
            shape = tuple(alloc.tensor_shape)
            dtype = mybir.dt.np(alloc.dtype)
            out_names.append(name)
            out_avals.append(jax.core.ShapedArray(shape, dtype))
            zero_outs.append(np.zeros(shape, dtype))
    n_params = len(in_names)
    n_outs = len(out_avals)
    all_in = in_names + out_names + ([partition_name] if partition_name else [])

    def _body(*args):
        operands = list(args)
        if partition_name is not None:
            operands.append(bass2jax.partition_id_tensor())
        outs = bass2jax._bass_exec_p.bind(
            *operands, out_avals=tuple(out_avals), in_names=tuple(all_in),
            out_names=tuple(out_names), lowering_input_output_aliases=(),
            sim_require_finite=False, sim_require_nnan=False, nc=nc)
        return tuple(outs)

    devices = jax.devices()[:n_cores]
    mesh = Mesh(np.asarray(devices), ("core",))
    in_specs = (PartitionSpec("core"),) * (n_params + n_outs)
    out_specs = (PartitionSpec("core"),) * n_outs
    jit_fn = jax.jit(
        shard_map(_body, mesh=mesh, in_specs=in_specs, out_specs=out_specs,
                  check_rep=False), keep_unused=True)

    def run(in_maps):
        per_core = [[np.asarray(m[n]) for n in in_names] for m in in_maps]
        args = [np.concatenate([per_core[c][i] for c in range(n_cores)], axis=0)
                for i in range(n_params)]
        args += [np.zeros((n_cores * z.shape[0], *z.shape[1:]), z.dtype)
                 for z in zero_outs]
        outs = [np.asarray(o) for o in jit_fn(*args)]
        return [{n: outs[i].reshape(n_cores, *out_avals[i].shape)[c]
                 for i, n in enumerate(out_names)}
                for c in range(n_cores)], jit_fn, args

    return run


_BUILT = {}


def _get_runner(kp):
    key = kp.tobytes()
    if key not in _BUILT:
        nc = build_bass(kp)
        _BUILT[key] = _make_runner(nc, N_CORES)
    return _BUILT[key]


def _host_prep(query_points, support_points, support_features,
               neighbor_indices, weights, bias, kernel_points):
    qp = np.asarray(query_points, np.float32)
    sp = np.asarray(support_points, np.float32)
    sf = np.asarray(support_features, np.float32)
    ni = np.asarray(neighbor_indices)
    ni = np.clip(ni, 0, M - 1).astype(np.int16)
    w = np.ascontiguousarray(np.asarray(weights, np.float32))
    bias = np.asarray(bias, np.float32).reshape(C_OUT, 1)

    mask120 = np.zeros((128, 120), np.float32)
    for q in range(8):
        mask120[q * 16:(q + 1) * 16, q * 15:(q + 1) * 15] = 1.0
    mask16 = np.zeros((128, 8), np.float32)
    for q in range(8):
        mask16[q * 16:(q + 1) * 16, q] = 1.0
    ident = np.eye(128, dtype=np.float32)
    ones1 = np.ones((1, 128), np.float32)
    kpv = np.asarray(kernel_points, np.float32)
    kpb = np.zeros((128, 48), np.float32)
    for p in range(P):
        for d in range(3):
            kpb[:, 3 * p + d] = -kpv[p, d]
    kpb[:, 45] = 1e-10
    kpb[:, 46] = -1.0 / SIGMA

    in_maps = []
    for c in range(N_CORES):
        b, half = divmod(c, 2)
        n0 = half * NQ_CORE
        idx = ni[b, n0:n0 + NQ_CORE, :].reshape(NK_CORE)
        # chunk order: idx j in chunk -> partition j%16 (k), col j//16;
        # stream order is already (query, k) = natural
        idx_l = idx.reshape(NK_CORE // 16, 16).T          # [16, NK/16]
        idx_l = np.tile(idx_l, (8, 1))                    # [128, NK/16]
        qrep = np.repeat(qp[b, n0:n0 + NQ_CORE, :], K, axis=0)  # [NK, 3]
        qrep = qrep.reshape(NK_CORE // 128, 128, 3).transpose(1, 0, 2)
        qrep = np.ascontiguousarray(qrep)
        in_maps.append({
            "sfeat": sf[b], "spts": sp[b], "qrep": qrep,
            "idx": np.ascontiguousarray(idx_l),
            "w": w, "bias": bias, "mask120": mask120, "mask16": mask16,
            "ident": ident, "ones1": ones1, "kpb": kpb,
            "onesc": np.ones((128, 1), np.float32),
        })
    return in_maps


# ===========================================================================
# Sparse path: kw = relu(1 - d/sigma) is ~99.99% zero for these inputs
# (support/query points uniform in [0,1]^3, sigma=0.03). Host finds a
# conservative SUPERSET of candidate (query, neighbor) pairs by integer
# cell binning (no float math decides values, only candidate pruning; any
# pair within reach of any kernel point is provably included). The device
# gathers those pairs' coords + features, computes exact kw and the two
# einsums for just those pairs, and scatter-adds into the bias-prefilled
# output. Falls back to the dense kernel when candidates exceed CAP.
# ===========================================================================
CAP = 5120          # static per-core candidate-pair capacity (40 blocks)
GRID = 128          # cells per axis for host binning
NBLK = CAP // 128
TRASH = NQ_CORE     # out_t row 8192 = trash for pad/unused slots


def build_sparse(reps=0, skip=()):
    sk = set(skip)
    nc = bass.Bass(dynamic_dma_scratch_size=32768, num_swdge_queues=4)

    ftab_in = nc.dram_tensor("ftab", [M, 128], F16, kind="ExternalInput")
    qsel_in = nc.dram_tensor("qsel", [128, (CAP // 128) * 4], F32,
                             kind="ExternalInput")
    wcat_in = nc.dram_tensor("wcat", [128, 2048], F16, kind="ExternalInput")
    kpcat_in = nc.dram_tensor("kpcat", [128, 48], F32, kind="ExternalInput")
    seg_in = nc.dram_tensor("seg", [128, CAP], F16, kind="ExternalInput")
    ident_in = nc.dram_tensor("ident16", [128, 128], F16, kind="ExternalInput")
    midx_in = nc.dram_tensor("midx", [128, CAP // 16], I16, kind="ExternalInput")
    nscidx_in = nc.dram_tensor("nscidx", [128, CAP // 16], I16, kind="ExternalInput")
    out_t = nc.dram_tensor("out", [NQ_CORE + 1, C_OUT], F32, kind="ExternalOutput")

    nc.gpsimd.load_library(library_config.mlp)

    with TileContext(nc) as tc:
        with tc.tile_pool(name="const", bufs=1) as cpool, \
             tc.tile_pool(name="gath", bufs=1) as gpool, \
             tc.tile_pool(name="work", bufs=2) as wpool, \
             tc.tile_pool(name="psf", bufs=2, space="PSUM") as psfpool, \
             tc.tile_pool(name="pst", bufs=2, space="PSUM") as pstpool, \
             tc.tile_pool(name="ps2", bufs=2, space="PSUM") as ps2pool:
            wcat_t = cpool.tile([128, 2048], F16, tag="wcat")
            nc.sync.dma_start(wcat_t[:], wcat_in[:])
            kpcat_t = cpool.tile([128, 48], F32, tag="kpcat")
            nc.sync.dma_start(kpcat_t[:], kpcat_in[:])
            seg_t = cpool.tile([128, CAP], F16, tag="seg")
            nc.sync.dma_start(seg_t[:], seg_in[:])
            ident_t = cpool.tile([128, 128], F16, tag="ident16")
            nc.sync.dma_start(ident_t[:], ident_in[:])
            midx_t = cpool.tile([128, CAP // 16], I16, tag="midx")
            nc.sync.dma_start(midx_t[:], midx_in[:])

            nscidx_t = cpool.tile([128, CAP // 16], I16, tag="nscidx")
            nc.sync.dma_start(nscidx_t[:], nscidx_in[:])
            nreg = nc.gpsimd.to_reg(CAP)
            greg = nc.gpsimd.to_reg(1024)
            done_sems = [nc.alloc_semaphore(f"scat_done{q}") for q in range(4)]
            swctr = [0]

            def swq():
                swctr[0] += 1
                return 0

            import contextlib
            loop_cm = tc.For_i(0, reps, 1) if reps else contextlib.nullcontext()
            with loop_cm:
                # gathers: fT (transpose mode), combined row (feats+coords),
                # query row
                GCH = 1024  # indices per dma_gather call
                NCH = CAP // GCH             # chunks
                BPC = GCH // 128             # blocks per chunk
                qsb = gpool.tile([128, NBLK, 4], F32, tag="qsb")
                nc.sync.dma_start(qsb[:], qsel_in[:].rearrange(
                    "a (b c) -> a b c", c=4))
                ssb_l, ftg_l, kwt_l = [], [], []
                for g in range(NCH):
                    ssb = gpool.tile([128, BPC, 64], F32, tag=f"ssb{g}")
                    if "ssb" in sk:
                        nc.vector.memset(ssb[:], 0.0)
                    else:
                        nc.gpsimd.dma_gather(
                            ssb[:], ftab_in[:].bitcast(F32),
                            midx_t[:, g * GCH // 16:(g + 1) * GCH // 16],
                            GCH, greg, 64, queue_num=swq())
                    ssb_l.append(ssb)
                    # fT via PE transpose, one 128-entry block at a time
                    ftg = gpool.tile([128, GCH], F16, tag=f"ftg{g}")
                    ssb16 = ssb[:].bitcast(F16)      # [128, BPC, 128]
                    for cc in range(BPC):
                        psumT = pstpool.tile([64, 128], F16, tag="pst")
                        nc.tensor.transpose(
                            psumT[:],
                            bass.AP(ssb16.tensor, ssb16.offset + cc * 128,
                                    [ssb16.ap[0], [1, 64]]),
                            ident_t[:])
                        nc.scalar.copy(
                            ap_part(ftg[:], 0, 64, cc * 128, [[1, 128]]),
                            psumT[:])
                    ftg_l.append(ftg)
                    # rel = s - q; kw = relu(1 - sqrt(d2)/sigma)
                    rel = gpool.tile([128, BPC, 3], F32, tag=f"rel{g}")
                    nc.vector.tensor_tensor(
                        out=rel[:],
                        in0=ap_view(ssb[:], 32, [[64, BPC], [1, 3]]),
                        in1=ap_view(qsb[:], g * BPC * 4, [[4, BPC], [1, 3]]),
                        op=mybir.AluOpType.subtract)
                    diff = gpool.tile([128, BPC * 45], F32, tag=f"diff{g}")
                    nc.vector.tensor_tensor(
                        out=ap_view(diff[:], 0, [[45, BPC], [3, P], [1, 3]]),
                        in0=ap_view(rel[:], 0, [[3, BPC], [0, P], [1, 3]]),
                        in1=ap_view(kpcat_t[:], 0, [[0, BPC], [3, P], [1, 3]]),
                        op=mybir.AluOpType.subtract)
                    nc.scalar.activation(diff[:], diff[:],
                                         mybir.ActivationFunctionType.Square,
                                         bias=0.0, scale=1.0)
                    kwt = gpool.tile([128, BPC, 16], F32, tag=f"kw{g}")
                    nc.vector.memset(kwt[:], 0.0)
                    nc.vector.tensor_reduce(
                        out=ap_view(kwt[:], 0, [[16, BPC], [1, P]]),
                        in_=ap_view(diff[:], 0, [[45, BPC], [3, P], [1, 3]]),
                        axis=mybir.AxisListType.X, op=mybir.AluOpType.add)
                    nc.scalar.activation(kwt[:], kwt[:],
                                         mybir.ActivationFunctionType.Sqrt,
                                         bias=kpcat_t[:, 45:46], scale=1.0)
                    nc.scalar.activation(kwt[:], kwt[:],
                                         mybir.ActivationFunctionType.Relu,
                                         bias=1.0, scale=kpcat_t[:, 46:47])
                    kwt_l.append(kwt)

                scat = gpool.tile([128, NBLK, C_OUT], F32, tag="scat")
                lp = nc.allow_low_precision(
                    reason="f16 weighted intermediates; validated vs "
                           "reference at 3e-4 rel err")
                lp.__enter__()
                for c in range(NBLK):
                    wtdm = wpool.tile([128, 2048], F16, tag="wtdm")
                    for hw in range(2):  # p 0:8 | p 8:16 (slot 15 zero-W)
                        psumF = psfpool.tile([128, 1024], F32, tag="psf")
                        for k in range(2):
                            nc.tensor.matmul(
                                psumF[:, k * 512:(k + 1) * 512],
                                ap_part(ftg_l[c // BPC][:], 0, C_IN,
                                        (c % BPC) * 128, [[1, 128]]),
                                ap_part(wcat_t[:], 0, C_IN,
                                        hw * 1024 + k * 512, [[1, 512]]),
                                start=True, stop=True)
                        fw16 = wpool.tile([128, 1024], F16, tag="fw16")
                        nc.scalar.copy(fw16[:], psumF[:])
                        nc.vector.tensor_tensor(
                            out=ap_view(wtdm[:], hw * 1024,
                                        [[C_OUT, 8], [1, C_OUT]]),
                            in0=ap_view(fw16[:], 0, [[C_OUT, 8], [1, C_OUT]]),
                            in1=ap_view(kwt_l[c // BPC][:],
                                        (c % BPC) * 16 + hw * 8,
                                        [[1, 8], [0, C_OUT]]),
                            op=mybir.AluOpType.mult)
                    tr1 = wpool.tile([128, 1024], F16, tag="tr1")
                    nc.vector.tensor_tensor(
                        out=tr1[:], in0=wtdm[:, 0:1024], in1=wtdm[:, 1024:2048],
                        op=mybir.AluOpType.add)
                    tr2 = wpool.tile([128, 512], F16, tag="tr2")
                    nc.vector.tensor_tensor(
                        out=tr2[:], in0=tr1[:, 0:512], in1=tr1[:, 512:1024],
                        op=mybir.AluOpType.add)
                    tr3 = wpool.tile([128, 256], F16, tag="tr3")
                    nc.vector.tensor_tensor(
                        out=tr3[:], in0=tr2[:, 0:256], in1=tr2[:, 256:512],
                        op=mybir.AluOpType.add)
                    ct = wpool.tile([128, C_OUT], F16, tag="ct")
                    nc.vector.tensor_tensor(
                        out=ct[:], in0=tr3[:, 0:128], in1=tr3[:, 128:256],
                        op=mybir.AluOpType.add)
                    psum2 = ps2pool.tile([128, C_OUT], F32, tag="ps2")
                    nc.tensor.matmul(psum2[:], seg_t[:, c * 128:(c + 1) * 128],
                                     ct[:], start=True, stop=True)
                    nc.scalar.copy(
                        ap_view(scat[:], c * C_OUT, [[1, C_OUT]]), psum2[:])

                lp.__exit__(None, None, None)
                if "scatter" not in sk:
                    qcnt = [0, 0, 0, 0]
                    for g in range(CAP // GCH):
                        q = 0
                        nc.gpsimd.dma_scatter_add(
                            out_t[:],
                            ap_view(scat[:], g * (GCH // 128) * C_OUT,
                                    [[C_OUT, GCH // 128], [1, C_OUT]]),
                            nscidx_t[:, g * GCH // 16:(g + 1) * GCH // 16],
                            GCH, greg, C_OUT,
                            queue_num=q).then_inc(done_sems[q], 16)
                        qcnt[q] += 16
                    for q in range(4):
                        if qcnt[q]:
                            nc.gpsimd.wait_ge(done_sems[q], qcnt[q])
                else:
                    nc.sync.dma_start(out_t[0:128, :],
                                      ap_view(scat[:], 0, [[1, C_OUT]]))
    return nc


def _make_runner_sparse(nc, n_cores):
    bass2jax.install_neuronx_cc_hook()
    from jax.sharding import Mesh, PartitionSpec
    from jax.experimental.shard_map import shard_map

    partition_name = nc.partition_id_tensor.name if nc.partition_id_tensor else None
    in_names, out_names, out_avals = [], [], []
    for alloc in nc.m.functions[0].allocations:
        if not isinstance(alloc, mybir.MemoryLocationSet):
            continue
        name = alloc.memorylocations[0].name
        if alloc.kind == "ExternalInput":
            if name != partition_name:
                in_names.append(name)
        elif alloc.kind == "ExternalOutput":
            shape = tuple(alloc.tensor_shape)
            dtype = mybir.dt.np(alloc.dtype)
            out_names.append(name)
            out_avals.append(jax.core.ShapedArray(shape, dtype))
    n_params = len(in_names)
    n_outs = len(out_avals)
    all_in = in_names + out_names + ([partition_name] if partition_name else [])

    def _body(*args):
        operands = list(args)
        if partition_name is not None:
            operands.append(bass2jax.partition_id_tensor())
        outs = bass2jax._bass_exec_p.bind(
            *operands, out_avals=tuple(out_avals), in_names=tuple(all_in),
            out_names=tuple(out_names), lowering_input_output_aliases=(),
            sim_require_finite=False, sim_require_nnan=False, nc=nc)
        return tuple(outs)

    devices = jax.devices()[:n_cores]
    mesh = Mesh(np.asarray(devices), ("core",))
    in_specs = (PartitionSpec("core"),) * (n_params + n_outs)
    out_specs = (PartitionSpec("core"),) * n_outs
    donate = tuple(range(n_params, n_params + n_outs))
    jit_fn = jax.jit(
        shard_map(_body, mesh=mesh, in_specs=in_specs, out_specs=out_specs,
                  check_rep=False), donate_argnums=donate, keep_unused=True)

    def run(in_maps, out_prefills):
        per_core = [[np.asarray(m[n]) for n in in_names] for m in in_maps]
        args = [np.concatenate([per_core[c][i] for c in range(n_cores)], axis=0)
                for i in range(n_params)]
        args += [np.concatenate([np.asarray(p[n]) for p in out_prefills], axis=0)
                 for n in out_names]
        outs = [np.asarray(o) for o in jit_fn(*args)]
        return [{n: outs[i].reshape(n_cores, *out_avals[i].shape)[c]
                 for i, n in enumerate(out_names)}
                for c in range(n_cores)], jit_fn, args

    return run


def _get_runner_sparse():
    if "sparse" not in _BUILT:
        nc = build_sparse()
        _BUILT["sparse"] = _make_runner_sparse(nc, N_CORES)
    return _BUILT["sparse"]


def _wrap16(vals, pad_val, dtype=np.int16):
    """List -> [128, CAP//16] wrapped (entry j at [j%16, j//16]), replicated
    across the 8 gpsimd cores."""
    buf = np.full(CAP, pad_val, dtype)
    buf[:len(vals)] = vals
    w = buf.reshape(CAP // 16, 16).T          # [16, CAP//16]
    return np.ascontiguousarray(np.tile(w, (8, 1)))


def _host_prep_sparse(qp, sp, sf, ni, w, bias_v, kpv):
    """Returns (in_maps, out_prefills) or None if candidates exceed CAP."""
    # conservative candidate radius: a hit needs |s - q| < sigma + max|kp|
    rmax = SIGMA + float(np.sqrt((kpv * kpv).sum(axis=1)).max())
    t_cell = (rmax * GRID + math.sqrt(3.0)) ** 2
    scell = np.clip((sp * GRID).astype(np.int32), 0, GRID - 1)
    qcell = np.clip((qp * GRID).astype(np.int32), 0, GRID - 1)

    wcat = np.zeros((128, 2048), np.float16)
    wcat[:C_IN, :P * C_OUT] = (np.transpose(w, (1, 0, 2)) / 16.0
                               ).reshape(C_IN, -1)
    wcat[C_IN:] = wcat[:C_IN]
    kpcat = np.zeros((128, 48), np.float32)
    kpcat[:, :45] = kpv.reshape(1, 45)
    kpcat[:, 45] = 1e-10
    kpcat[:, 46] = -1.0 / SIGMA

    in_maps, out_prefills = [], []
    for c in range(N_CORES):
        b, half = divmod(c, 2)
        n0 = half * NQ_CORE
        nib = ni[b, n0:n0 + NQ_CORE]
        dc = scell[b][nib] - qcell[b, n0:n0 + NQ_CORE, None, :]
        d2 = (dc.astype(np.int64) ** 2).sum(axis=2)
        nn, kk = np.nonzero(d2 <= t_cell)      # sorted by n (row-major)
        mm = nib[nn, kk]

        # pack into 128-entry blocks; no query group spans a block boundary
        m_list = np.zeros(CAP, np.int16)
        n_list = np.zeros(CAP, np.int16)
        seg = np.zeros((128, CAP), np.float16)
        sc_idx = np.full(CAP, TRASH, np.int16)
        uniq, counts = np.unique(nn, return_counts=True)
        t = 0            # global entry cursor
        gi = 0           # group cursor
        ok = True
        off = 0          # start of each group's pairs in nn/kk
        for g in range(len(uniq)):
            cnt = counts[g]
            blk, pos = divmod(t, 128)
            if pos + cnt > 128:                # pad to next block
                t = (blk + 1) * 128
                blk, pos = blk + 1, 0
            if t + cnt > CAP:
                ok = False
                break
            d = 127                            # d-slot for this group
            # d slots allocated in order of first use within the block
            # (track per-block next free slot)
            m_list[t:t + cnt] = mm[off:off + cnt]
            n_list[t:t + cnt] = uniq[g]
            t += cnt
            off += cnt
        if not ok:
            return None
        # second pass: assign d-slots and seg/sc_idx now that layout is fixed
        seg[:] = 0
        sc_idx[:] = TRASH
        blk_next = np.zeros(NBLK, np.int32)
        t = 0
        off = 0
        for g in range(len(uniq)):
            cnt = counts[g]
            blk, pos = divmod(t, 128)
            if pos + cnt > 128:
                t = (blk + 1) * 128
                blk, pos = blk + 1, 0
            d = blk_next[blk]
            blk_next[blk] += 1
            seg[pos:pos + cnt, blk * 128 + d] = 1.0
            sc_idx[blk * 128 + d] = uniq[g]
            t += cnt
            off += cnt
        # pad entries (between groups / tail): m=0, n=0 gathers; their seg
        # column stays 0 -> contribute nothing; unused d-slots scatter to
        # TRASH row.

        ftab = np.zeros((M, 128), np.float16)
        ftab[:, :C_IN] = sf[b]
        ftab.view(np.float32)[:, 32:35] = sp[b]
        qsel = np.zeros((CAP, 4), np.float32)
        qsel[:, :3] = qp[b, n0 + n_list.astype(np.int64)]
        qsel = np.ascontiguousarray(
            qsel.reshape(NBLK, 128, 4).transpose(1, 0, 2)).reshape(128, -1)
        in_maps.append({
            "ftab": ftab, "qsel": qsel, "wcat": wcat, "kpcat": kpcat,
            "seg": seg, "ident16": np.eye(128, dtype=np.float16),
            "midx": _wrap16(m_list, 0),
            "nscidx": _wrap16(sc_idx, TRASH),
        })
        out_prefills.append({
            "out": np.tile(bias_v.reshape(1, C_OUT),
                           (NQ_CORE + 1, 1)).astype(np.float32)})
    return in_maps, out_prefills


def _kernel_dense(qp_raw, sp_raw, sf_raw, ni_raw, w_raw, bias_raw, kp_raw):
    kp = np.asarray(kp_raw, np.float32)
    run = _get_runner(kp)
    in_maps = _host_prep(qp_raw, sp_raw, sf_raw, ni_raw, w_raw, bias_raw,
                         kp_raw)
    results, _, _ = run(in_maps)
    out = np.zeros((B, N, C_OUT), np.float32)
    for c in range(N_CORES):
        b, half = divmod(c, 2)
        n0 = half * NQ_CORE
        out[b, n0:n0 + NQ_CORE, :] = results[c]["out"]
    return out


def kernel(query_points, support_points, support_features, neighbor_indices,
           weights, bias, kernel_points):
    qp = np.asarray(query_points, np.float32)
    sp = np.asarray(support_points, np.float32)
    sf = np.asarray(support_features, np.float32)
    ni = np.clip(np.asarray(neighbor_indices), 0, M - 1).astype(np.int32)
    w = np.asarray(weights, np.float32)
    bias_v = np.asarray(bias, np.float32)
    kpv = np.asarray(kernel_points, np.float32)

    prep = _host_prep_sparse(qp, sp, sf, ni, w, bias_v, kpv)
    if prep is None:
        return _kernel_dense(query_points, support_points, support_features,
                             neighbor_indices, weights, bias, kernel_points)
    in_maps, out_prefills = prep
    run = _get_runner_sparse()
    results, _, _ = run(in_maps, out_prefills)
    out = np.zeros((B, N, C_OUT), np.float32)
    for c in range(N_CORES):
        b, half = divmod(c, 2)
        n0 = half * NQ_CORE
        out[b, n0:n0 + NQ_CORE, :] = results[c]["out"][:NQ_CORE]

    # exact neighbor-count correction (reference divides by the number of
    # neighbors with nonzero features, clipped to >= 1; the device divides
    # by K=16). For randn features every row is nonzero, so cnt == 16 and
    # this is a no-op; handle degenerate inputs on host for full fidelity.
    row_nz = np.abs(sf).sum(axis=2) > 0          # [B, M]
    if not row_nz.all():
        z = row_nz.astype(np.float32)
        cnt = np.clip(
            z[np.arange(B)[:, None, None], ni].sum(axis=2), 1.0, None)
        out = (out - bias_v) * (16.0 / cnt)[..., None] + bias_v
    return out



# revision 37
# speedup vs baseline: 1.0287x; 1.0200x over previous
"""KPConv (nn_KPConvFPN) Trainium2 Bass kernel — sparse candidate-pair design.

kw = relu(1 - |s[m] - q[n] - kp_p|/sigma) is ~97.6% zero for these inputs
(points uniform in [0,1]^3, sigma + max|kp| = 0.0825). The host finds a
conservative SUPERSET of candidate (query, neighbor) pairs by integer cell
binning (GRID=128; any pair within reach of any kernel point is provably
included; no float math decides output values on the host). Per core
(batch b=c//2, query half c%2):

Device pipeline (CAP=5120 candidate pairs, 40 blocks of 128):
  1. Per 1024-pair chunk: SWDGE dma_gather of combined 256B rows
     [64 f16 feats | s-coords f32] from ftab; PE-transpose feats -> fT;
     DVE/ACT compute kw[t, p] for all 15 kernel points.
     (query coords arrive pre-gathered from host as qsel, like the dense
     kernel's qrep.)
  2. Per 128-pair block: 4 PE matmuls fW = fT @ [W_0|..|W_15]/16 (f16,
     2048 psum cols); DVE multiply by kw broadcast over C_out; binary-tree
     add over the 16 p-slots -> ct[t, 128].
  3. Segment matmul psum2[d, o] = seg[t, d]^T @ ct (host-built 0/1 seg
     matrix; groups pairs of the same query; pads/unused -> trash slot),
     so every output row is scattered EXACTLY once (dma_scatter_add loses
     updates on duplicate rows -- measured).
  4. dma_scatter_add rows into the bias-prefilled donated output buffer
     (row 8192 = trash row for pad slots).

Falls back to the dense kernel (build_bass below) when candidates exceed
CAP. The reference divides by the count of neighbors with nonzero
features; for randn features that is always K=16 (folded into W/16); the
degenerate case is corrected exactly on the host.
"""
import json
import math
import os

SKIP = set()

import numpy as np
import jax

import concourse.bass as bass
import concourse.mybir as mybir
from concourse.tile import TileContext
from concourse import library_config
from concourse import bass2jax

F32 = mybir.dt.float32
F16 = mybir.dt.float16
I16 = mybir.dt.int16

B, N, M, K = 4, 16384, 16384, 16
C_IN, C_OUT, P = 64, 128, 15
SIGMA = 0.03
N_CORES = 8
NQ_CORE = N // 2            # 8192 queries per core
NK_CORE = NQ_CORE * K       # 131072 gathered rows per core
ST_Q = 512                  # queries per supertile
N_ST = NQ_CORE // ST_Q      # 16
KW_ST = 2                   # supertiles per kw group
G_ST = ST_Q * K // 128      # 64 g-cols per supertile
ROW16 = 128                 # fp16 units per table row (256B)

# ---------------------------------------------------------------------------
# walrus workaround: this nix walrus build supports ONE sync-wait per
# instruction; split extra waits onto NoOps inserted before the offender
# (same-engine program order preserves semantics). Also run
# codegen_inst_isa_subclasses (Bacc does; raw Bass doesn't) so extended
# instructions get their ISA bytes.
_orig_to_json_bytes = bass.Bass.to_json_bytes


def _fix_block(bb, ctr):
    insts = bb.get("instructions")
    if not isinstance(insts, list):
        return
    new = []
    for inst in insts:
        si = inst.get("sync_info")
        ow = si.get("on_wait") if isinstance(si, dict) else None
        if ow and len(ow) > 1:
            for w in ow[:-1]:
                ctr[0] += 1
                nop = {"engine": inst["engine"], "ins": [], "outs": [],
                       "name": f"I-wsplit-{ctr[0]}", "opcode": "NoOp",
                       "sync_info": {"on_update": [], "on_wait": [w]},
                       "text_hint": "wsplit"}
                if "debug" in inst:
                    nop["debug"] = inst["debug"]
                new.append(nop)
            si["on_wait"] = [ow[-1]]
        new.append(inst)
    bb["instructions"] = new


def _walk(o, ctr):
    if isinstance(o, dict):
        if isinstance(o.get("instructions"), list):
            _fix_block(o, ctr)
        for v in o.values():
            _walk(v, ctr)
    elif isinstance(o, list):
        for v in o:
            _walk(v, ctr)


def _to_json_bytes_split(self):
    mybir.codegen_inst_isa_subclasses(self)
    raw = _orig_to_json_bytes(self)
    d = json.loads(raw)
    ctr = [0]
    _walk(d, ctr)
    return json.dumps(d).encode()


bass.Bass.to_json_bytes = _to_json_bytes_split


def ap_view(t_ap, extra_offset, dims):
    """AP over tile t_ap with explicit free dims [[step, count], ...]
    (steps in elements); partition dim is taken from the tile."""
    return bass.AP(t_ap.tensor, t_ap.offset + extra_offset,
                   [t_ap.ap[0]] + list(dims))


def ap_part(t_ap, pstart, pcount, extra_offset, dims):
    pstep = t_ap.ap[0][0]
    return bass.AP(t_ap.tensor, t_ap.offset + pstart * pstep + extra_offset,
                   [[pstep, pcount]] + list(dims))


def build_bass(kp, reps=0, skip=()):
    global SKIP
    SKIP = set(skip)
    """kp: (15, 3) float32 numpy kernel points (runtime values baked)."""
    kpsq = (kp * kp).sum(axis=1)  # |kp_p|^2
    nc = bass.Bass(dynamic_dma_scratch_size=32768, num_swdge_queues=4)

    feats_in = nc.dram_tensor("sfeat", [M, C_IN], F32, kind="ExternalInput")
    pts_in = nc.dram_tensor("spts", [M, 3], F32, kind="ExternalInput")
    qrep_in = nc.dram_tensor("qrep", [128, NK_CORE // 128, 3], F32,
                             kind="ExternalInput")
    idx_in = nc.dram_tensor("idx", [128, NK_CORE // 16], I16,
                            kind="ExternalInput")
    w_in = nc.dram_tensor("w", [P, C_IN, C_OUT], F32, kind="ExternalInput")
    bias_in = nc.dram_tensor("bias", [C_OUT, 1], F32, kind="ExternalInput")
    mask120_in = nc.dram_tensor("mask120", [128, 120], F32, kind="ExternalInput")
    mask16_in = nc.dram_tensor("mask16", [128, 8], F32, kind="ExternalInput")
    ident_in = nc.dram_tensor("ident", [128, 128], F32, kind="ExternalInput")
    ones1_in = nc.dram_tensor("ones1", [1, 128], F32, kind="ExternalInput")
    kpb_in = nc.dram_tensor("kpb", [128, 48], F32, kind="ExternalInput")
    onesc_in = nc.dram_tensor("onesc", [128, 1], F32, kind="ExternalInput")
    out_t = nc.dram_tensor("out", [NQ_CORE, C_OUT], F32, kind="ExternalOutput")
    table = nc.dram_tensor("table", [M, ROW16], F16, kind="Internal")

    # library load as raw preamble (before Tile scheduling) so it is
    # guaranteed to precede every dma_gather on the Pool engine.
    nc.gpsimd.load_library(library_config.mlp)

    with TileContext(nc) as tc:
        with tc.tile_pool(name="const", bufs=1) as cpool, \
             tc.tile_pool(name="build", bufs=1) as bpool, \
             tc.tile_pool(name="gath", bufs=2) as gpool, \
             tc.tile_pool(name="kwp", bufs=2) as kwpool, \
             tc.tile_pool(name="kbd", bufs=1) as kbpool, \
             tc.tile_pool(name="wt", bufs=1) as wtpool, \
             tc.tile_pool(name="sm", bufs=3) as smpool, \
             tc.tile_pool(name="fin", bufs=2) as fpool, \
             tc.tile_pool(name="ps1", bufs=2, space="PSUM") as ps1pool, \
             tc.tile_pool(name="ps2", bufs=2, space="PSUM") as ps2pool, \
             tc.tile_pool(name="ps3", bufs=1, space="PSUM") as ps3pool:

            # ---- constants ----
            wp_t = cpool.tile([C_IN, P * C_OUT], F32, tag="wp")
            nc.sync.dma_start(
                wp_t[:].rearrange("c (p o) -> c p o", p=P),
                w_in[:].rearrange("p c o -> c p o"))
            bias_t = cpool.tile([C_OUT, 1], F32, tag="bias")
            nc.sync.dma_start(bias_t[:], bias_in[:])
            mask120_t = cpool.tile([128, 120], F32, tag="m120")
            nc.sync.dma_start(mask120_t[:], mask120_in[:])
            mask16_t = cpool.tile([128, 8], F32, tag="m16")
            nc.sync.dma_start(mask16_t[:], mask16_in[:])
            ident_t = cpool.tile([128, 128], F32, tag="ident")
            nc.sync.dma_start(ident_t[:], ident_in[:])
            ones1_t = cpool.tile([1, 128], F32, tag="ones1")
            nc.sync.dma_start(ones1_t[:], ones1_in[:])
            kpb_t = cpool.tile([128, 48], F32, tag="kpb")
            nc.sync.dma_start(kpb_t[:], kpb_in[:])
            onesc_t = cpool.tile([128, 1], F32, tag="onesc")
            nc.sync.dma_start(onesc_t[:], onesc_in[:])
            nidx_reg = nc.gpsimd.to_reg(1024)

            # ---- 1. combined table build (8 chunks x 2048 rows) ----
            import contextlib
            loop_cm = tc.For_i(0, reps, 1) if reps else contextlib.nullcontext()
            with loop_cm:
                _table_build(nc, tc, bpool, feats_in, pts_in, table)
                _main_pipeline(nc, tc, gpool, kwpool, kbpool, wtpool, smpool,
                               fpool, ps1pool, ps2pool, ps3pool, kp,
                               qrep_in, idx_in, out_t, table, wp_t, bias_t,
                               mask120_t, mask16_t, ident_t, ones1_t, kpb_t,
                               onesc_t, nidx_reg)
    return nc


def _table_build(nc, tc, bpool, feats_in, pts_in, table):
            for ch in range(8):
                m0 = ch * 2048
                fsb = bpool.tile([128, 16, C_IN], F32, tag="fsb")
                nc.sync.dma_start(
                    fsb[:],
                    feats_in[m0:m0 + 2048, :].rearrange(
                        "(a p) c -> p a c", p=128))
                psb = bpool.tile([128, 16, 3], F32, tag="psb")
                nc.sync.dma_start(
                    psb[:],
                    pts_in[m0:m0 + 2048, :].rearrange(
                        "(a p) c -> p a c", p=128))
                st16 = bpool.tile([128, 16, ROW16], F16, tag="st16")
                nc.vector.tensor_copy(st16[:, :, 0:C_IN], fsb[:])
                stv32 = st16[:].bitcast(F32)  # [128, 16, 64] f32 view
                # aux: sx sy sz at f32-cols 32..34
                nc.vector.tensor_copy(
                    bass.AP(stv32.tensor, stv32.offset + 32,
                            [stv32.ap[0], [64, 16], [1, 3]]),
                    psb[:])
                # |s|^2 at f32-col 35
                psq = bpool.tile([128, 16, 3], F32, tag="psq")
                nc.vector.tensor_tensor(out=psq[:], in0=psb[:], in1=psb[:],
                                        op=mybir.AluOpType.mult)
                nc.vector.tensor_reduce(
                    out=bass.AP(stv32.tensor, stv32.offset + 35,
                                [stv32.ap[0], [64, 16], [1, 1]]),
                    in_=psq[:], axis=mybir.AxisListType.X,
                    op=mybir.AluOpType.add)
                # z at f32-col 36: (sum_c |f|) > 0
                zred = bpool.tile([128, 16, 1], F32, tag="zred")
                nc.vector.tensor_reduce(out=zred[:], in_=fsb[:],
                                        axis=mybir.AxisListType.X,
                                        op=mybir.AluOpType.add,
                                        apply_absolute_value=True)
                nc.vector.tensor_scalar(
                    out=bass.AP(stv32.tensor, stv32.offset + 36,
                                [stv32.ap[0], [64, 16], [1, 1]]),
                    in0=zred[:], scalar1=0.0, scalar2=None,
                    op0=mybir.AluOpType.is_gt)
                nc.sync.dma_start(
                    table[m0:m0 + 2048, :].rearrange("(a p) c -> p a c",
                                                     p=128),
                    st16[:])


def _main_pipeline(nc, tc, gpool, kwpool, kbpool, wtpool, smpool, fpool,
                   ps1pool, ps2pool, ps3pool, kp, qrep_in, idx_in, out_t,
                   table, wp_t, bias_t, mask120_t, mask16_t, ident_t,
                   ones1_t, kpb_t, onesc_t, nidx_reg):
            for kg in range(N_ST // KW_ST):  # kw group of 2 supertiles
                GQ = KW_ST * ST_Q            # 1024 queries
                GG = KW_ST * G_ST            # 128 g-cols
                gt = gpool.tile([128, GG, ROW16], F16, tag="gath")
                gt32 = gt[:].bitcast(F32)  # [128, GG, 64] f32 view
                # gathers: 16 chunks of 1024 idx
                if "gather" in SKIP:
                    nc.vector.memset(gt[:], 0.0)
                for g in range(GG // 8):
                    if "gather" in SKIP:
                        break
                    idxsl = smpool.tile([128, 64], I16, tag="idxsl")
                    nc.sync.dma_start(
                        idxsl[:],
                        idx_in[:, (kg * 16 + g) * 64:(kg * 16 + g) * 64 + 64])
                    nc.gpsimd.dma_gather(
                        gt[:, g * 8:(g + 1) * 8, :], table[:], idxsl[:],
                        1024, nidx_reg, ROW16, queue_num=g % 4)
                # qrep slice
                qr = smpool.tile([128, GG, 3], F32, tag="qr")
                nc.sync.dma_start(qr[:], qrep_in[:, kg * GG:(kg + 1) * GG, :])
                # rel = s - q
                rel = smpool.tile([128, GG, 3], F32, tag="rel")
                nc.vector.tensor_tensor(
                    out=rel[:],
                    in0=ap_view(gt32, 32, [[64, GG], [1, 3]]),
                    in1=qr[:], op=mybir.AluOpType.subtract)
                # d2[p] = sum_dim (rel_dim - kp[p,dim])^2  (ACT squares, DVE adds)
                kwt = kwpool.tile([128, GG, P], F32, tag="kw")
                sq0 = smpool.tile([128, GG], F32, tag="sq0")
                sq1 = smpool.tile([128, GG], F32, tag="sq1")
                if "kw" in SKIP:
                    nc.vector.memset(kwt[:], 0.0)
                for p in range(P if "kw" not in SKIP else 0):
                    d2dst = ap_view(kwt[:], p, [[P, GG], [1, 1]])
                    nc.scalar.activation(
                        sq0[:], ap_view(rel[:], 0, [[3, GG], [1, 1]]),
                        mybir.ActivationFunctionType.Square,
                        bias=kpb_t[:, 3 * p:3 * p + 1], scale=1.0)
                    nc.scalar.activation(
                        sq1[:], ap_view(rel[:], 1, [[3, GG], [1, 1]]),
                        mybir.ActivationFunctionType.Square,
                        bias=kpb_t[:, 3 * p + 1:3 * p + 2], scale=1.0)
                    nc.vector.tensor_tensor(out=sq0[:], in0=sq0[:],
                                            in1=sq1[:],
                                            op=mybir.AluOpType.add)
                    nc.scalar.activation(
                        sq1[:], ap_view(rel[:], 2, [[3, GG], [1, 1]]),
                        mybir.ActivationFunctionType.Square,
                        bias=kpb_t[:, 3 * p + 2:3 * p + 3], scale=1.0)
                    nc.vector.tensor_tensor(out=d2dst, in0=sq0[:],
                                            in1=sq1[:],
                                            op=mybir.AluOpType.add)
                # kw = relu(1 - sqrt(d2 + 1e-10)/sigma), in place
                if "kw" in SKIP:
                    pass
                else:
                    nc.scalar.activation(kwt[:], kwt[:],
                                     mybir.ActivationFunctionType.Sqrt,
                                     bias=kpb_t[:, 45:46], scale=1.0)
                if "kw" not in SKIP:
                    nc.scalar.activation(kwt[:], kwt[:],
                                     mybir.ActivationFunctionType.Relu,
                                     bias=1.0, scale=kpb_t[:, 46:47])

                for sti in range(KW_ST):
                    st = kg * KW_ST + sti
                    # kwbd (2 half-ST TT ops): [128, (bl32, q8, p15)] fp16
                    kbd = kbpool.tile([128, 3840], F16, tag="kbd")
                    kbd2 = kbpool.tile([128, 3840], F16, tag="kbd2")
                    if "kwbd" in SKIP:
                        nc.vector.memset(kbd[:], 0.0)
                        nc.vector.memset(kbd2[:], 0.0)
                    for hf, kb in ((0, kbd), (1, kbd2)) if "kwbd" not in SKIP else ():
                        bl0 = sti * G_ST + hf * 32
                        nc.vector.tensor_tensor(
                            out=ap_view(kb[:], 0,
                                        [[120, 32], [15, 8], [1, 15]]),
                            in0=ap_view(kwt[:], bl0 * P,
                                        [[P, 32], [0, 8], [1, P]]),
                            in1=ap_view(mask120_t[:], 0,
                                        [[0, 32], [15, 8], [1, 15]]),
                            op=mybir.AluOpType.mult)
                    # einsum1: 64 blocks
                    wtt = wtpool.tile([64, 7680], F32, tag="wt")
                    if "e1" in SKIP:
                        nc.vector.memset(wtt[:], 0.0)
                    for bg in range(16 if "e1" not in SKIP else 0):  # bank groups of 4 blocks (32 q)
                        pse1 = ps1pool.tile([64, 480], F32, tag="pse1")
                        for j in range(4):
                            bl = bg * 4 + j          # block in supertile
                            blg = sti * G_ST + bl    # g-col in group tile
                            kb = kbd if bl < 32 else kbd2
                            kbl = bl % 32
                            nc.tensor.matmul(
                                pse1[:, j * 120:(j + 1) * 120],
                                ap_view(gt[:], blg * ROW16, [[1, C_IN]]),
                                ap_view(kb[:], kbl * 120, [[1, 120]]),
                                start=True, stop=True)
                        # evict (split DVE/ACT)
                        nc.vector.tensor_copy(
                            wtt[:, bg * 480:bg * 480 + 240],
                            pse1[:, 0:240])
                        nc.scalar.copy(
                            wtt[:, bg * 480 + 240:bg * 480 + 480],
                            pse1[:, 240:480])
                    # count row: zbd = z * mask16 -> ones-row matmul
                    zbd = smpool.tile([128, 512], F32, tag="zbd")
                    nc.vector.tensor_tensor(
                        out=zbd[:].rearrange("a (g j q) -> a g j q",
                                             g=16, j=4),
                        in0=ap_view(gt32, (sti * G_ST) * 64 + 36,
                                    [[256, 16], [64, 4], [0, 8]]),
                        in1=ap_view(mask16_t[:], 0,
                                    [[0, 16], [0, 4], [1, 8]]),
                        op=mybir.AluOpType.mult)
                    pscnt = ps3pool.tile([1, 512], F32, tag="pscnt")
                    nc.tensor.matmul(pscnt[:], onesc_t[:], zbd[:],
                                     start=True, stop=True)
                    cntinv = smpool.tile([1, 512], F32, tag="cntinv")
                    nc.vector.tensor_scalar(out=cntinv[:], in0=pscnt[:],
                                            scalar1=1.0, scalar2=None,
                                            op0=mybir.AluOpType.max)
                    nc.vector.reciprocal(out=cntinv[:], in_=cntinv[:])
                    psrep = ps3pool.tile([128, 512], F32, tag="psrep")
                    nc.tensor.matmul(psrep[:], ones1_t[:], cntinv[:],
                                     start=True, stop=True)
                    # note: psrep = cntinv^T replicated? see host mapping
                    cntrep = smpool.tile([128, 512], F32, tag="cntrep")
                    nc.vector.tensor_copy(cntrep[:], psrep[:])

                    # einsum2: out[o, s] accumulated over p
                    pse2 = ps2pool.tile([128, 512], F32, tag="pse2")
                    for p in range(P if "e2" not in SKIP else 1):
                        nc.tensor.matmul(
                            pse2[:],
                            ap_view(wp_t[:], p * C_OUT, [[1, C_OUT]]),
                            ap_view(wtt[:], p,
                                    [[480, 16], [120, 4], [15, 8]]),
                            start=(p == 0), stop=True)
                    # divide by count, add bias
                    e2sb = fpool.tile([128, 512], F32, tag="e2sb")
                    nc.vector.tensor_tensor(out=e2sb[:], in0=pse2[:],
                                            in1=cntrep[:],
                                            op=mybir.AluOpType.mult)
                    nc.vector.tensor_scalar(out=e2sb[:], in0=e2sb[:],
                                            scalar1=bias_t[:],
                                            scalar2=None,
                                            op0=mybir.AluOpType.add)
                    # transpose 4x128 cols and store
                    for t4 in range(4):
                        pstr = ps3pool.tile([128, 128], F32, tag="pstr")
                        nc.tensor.transpose(
                            pstr[:], e2sb[:, t4 * 128:(t4 + 1) * 128],
                            ident_t[:])
                        trsb = fpool.tile([128, 128], F32, tag="trsb")
                        nc.scalar.copy(trsb[:], pstr[:])
                        # e2 cols are n-linear: plain contiguous store
                        n0 = st * 512 + t4 * 128
                        nc.sync.dma_start(out_t[n0:n0 + 128, :], trsb[:])


def _make_runner(nc, n_cores):
    bass2jax.install_neuronx_cc_hook()
    from jax.sharding import Mesh, PartitionSpec
    from jax.experimental.shard_map import shard_map

    partition_name = nc.partition_id_tensor.name if nc.partition_id_tensor else None
    in_names, out_names, out_avals, zero_outs = [], [], [], []
    for alloc in nc.m.functions[0].allocations:
        if not isinstance(alloc, mybir.MemoryLocationSet):
            continue
        name = alloc.memorylocations[0].name
        if alloc.kind == "ExternalInput":
            if name != partition_name:
                in_names.append(name)
        elif alloc.kind == "ExternalOutput":
            shape = tuple(alloc.tensor_shape)
            dtype = mybir.dt.np(alloc.dtype)
            out_names.append(name)
            out_avals.append(jax.core.ShapedArray(shape, dtype))
            zero_outs.append(np.zeros(shape, dtype))
    n_params = len(in_names)
    n_outs = len(out_avals)
    all_in = in_names + out_names + ([partition_name] if partition_name else [])

    def _body(*args):
        operands = list(args)
        if partition_name is not None:
            operands.append(bass2jax.partition_id_tensor())
        outs = bass2jax._bass_exec_p.bind(
            *operands, out_avals=tuple(out_avals), in_names=tuple(all_in),
            out_names=tuple(out_names), lowering_input_output_aliases=(),
            sim_require_finite=False, sim_require_nnan=False, nc=nc)
        return tuple(outs)

    devices = jax.devices()[:n_cores]
    mesh = Mesh(np.asarray(devices), ("core",))
    in_specs = (PartitionSpec("core"),) * (n_params + n_outs)
    out_specs = (PartitionSpec("core"),) * n_outs
    jit_fn = jax.jit(
        shard_map(_body, mesh=mesh, in_specs=in_specs, out_specs=out_specs,
                  check_rep=False), keep_unused=True)

    def run(in_maps):
        per_core = [[np.asarray(m[n]) for n in in_names] for m in in_maps]
        args = [np.concatenate([per_core[c][i] for c in range(n_cores)], axis=0)
                for i in range(n_params)]
        args += [np.zeros((n_cores * z.shape[0], *z.shape[1:]), z.dtype)
                 for z in zero_outs]
        outs = [np.asarray(o) for o in jit_fn(*args)]
        return [{n: outs[i].reshape(n_cores, *out_avals[i].shape)[c]
                 for i, n in enumerate(out_names)}
                for c in range(n_cores)], jit_fn, args

    return run


_BUILT = {}


def _get_runner(kp):
    key = kp.tobytes()
    if key not in _BUILT:
        nc = build_bass(kp)
        _BUILT[key] = _make_runner(nc, N_CORES)
    return _BUILT[key]


def _host_prep(query_points, support_points, support_features,
               neighbor_indices, weights, bias, kernel_points):
    qp = np.asarray(query_points, np.float32)
    sp = np.asarray(support_points, np.float32)
    sf = np.asarray(support_features, np.float32)
    ni = np.asarray(neighbor_indices)
    ni = np.clip(ni, 0, M - 1).astype(np.int16)
    w = np.ascontiguousarray(np.asarray(weights, np.float32))
    bias = np.asarray(bias, np.float32).reshape(C_OUT, 1)

    mask120 = np.zeros((128, 120), np.float32)
    for q in range(8):
        mask120[q * 16:(q + 1) * 16, q * 15:(q + 1) * 15] = 1.0
    mask16 = np.zeros((128, 8), np.float32)
    for q in range(8):
        mask16[q * 16:(q + 1) * 16, q] = 1.0
    ident = np.eye(128, dtype=np.float32)
    ones1 = np.ones((1, 128), np.float32)
    kpv = np.asarray(kernel_points, np.float32)
    kpb = np.zeros((128, 48), np.float32)
    for p in range(P):
        for d in range(3):
            kpb[:, 3 * p + d] = -kpv[p, d]
    kpb[:, 45] = 1e-10
    kpb[:, 46] = -1.0 / SIGMA

    in_maps = []
    for c in range(N_CORES):
        b, half = divmod(c, 2)
        n0 = half * NQ_CORE
        idx = ni[b, n0:n0 + NQ_CORE, :].reshape(NK_CORE)
        # chunk order: idx j in chunk -> partition j%16 (k), col j//16;
        # stream order is already (query, k) = natural
        idx_l = idx.reshape(NK_CORE // 16, 16).T          # [16, NK/16]
        idx_l = np.tile(idx_l, (8, 1))                    # [128, NK/16]
        qrep = np.repeat(qp[b, n0:n0 + NQ_CORE, :], K, axis=0)  # [NK, 3]
        qrep = qrep.reshape(NK_CORE // 128, 128, 3).transpose(1, 0, 2)
        qrep = np.ascontiguousarray(qrep)
        in_maps.append({
            "sfeat": sf[b], "spts": sp[b], "qrep": qrep,
            "idx": np.ascontiguousarray(idx_l),
            "w": w, "bias": bias, "mask120": mask120, "mask16": mask16,
            "ident": ident, "ones1": ones1, "kpb": kpb,
            "onesc": np.ones((128, 1), np.float32),
        })
    return in_maps


# ===========================================================================
# Sparse path: kw = relu(1 - d/sigma) is ~99.99% zero for these inputs
# (support/query points uniform in [0,1]^3, sigma=0.03). Host finds a
# conservative SUPERSET of candidate (query, neighbor) pairs by integer
# cell binning (no float math decides values, only candidate pruning; any
# pair within reach of any kernel point is provably included). The device
# gathers those pairs' coords + features, computes exact kw and the two
# einsums for just those pairs, and scatter-adds into the bias-prefilled
# output. Falls back to the dense kernel when candidates exceed CAP.
# ===========================================================================
CAP = 5120          # static per-core candidate-pair capacity (40 blocks)
GRID = 128          # cells per axis for host binning
NBLK = CAP // 128
TRASH = NQ_CORE     # out_t row 8192 = trash for pad/unused slots


def build_sparse(reps=0, skip=()):
    sk = set(skip)
    nc = bass.Bass(dynamic_dma_scratch_size=32768, num_swdge_queues=4)

    ftab_in = nc.dram_tensor("ftab", [M, 128], F16, kind="ExternalInput")
    qsel_in = nc.dram_tensor("qsel", [128, (CAP // 128) * 4], F32,
                             kind="ExternalInput")
    wcat_in = nc.dram_tensor("wcat", [128, 2048], F16, kind="ExternalInput")
    kpcat_in = nc.dram_tensor("kpcat", [128, 48], F32, kind="ExternalInput")
    seg_in = nc.dram_tensor("seg", [128, CAP], F16, kind="ExternalInput")
    ident_in = nc.dram_tensor("ident16", [128, 128], F16, kind="ExternalInput")
    midx_in = nc.dram_tensor("midx", [128, CAP // 16], I16, kind="ExternalInput")
    nscidx_in = nc.dram_tensor("nscidx", [128, CAP // 16], I16, kind="ExternalInput")
    out_t = nc.dram_tensor("out", [NQ_CORE + 1, C_OUT], F32, kind="ExternalOutput")

    nc.gpsimd.load_library(library_config.mlp)

    with TileContext(nc) as tc:
        with tc.tile_pool(name="const", bufs=1) as cpool, \
             tc.tile_pool(name="gath", bufs=1) as gpool, \
             tc.tile_pool(name="work", bufs=2) as wpool, \
             tc.tile_pool(name="psf", bufs=2, space="PSUM") as psfpool, \
             tc.tile_pool(name="pst", bufs=2, space="PSUM") as pstpool, \
             tc.tile_pool(name="ps2", bufs=2, space="PSUM") as ps2pool:
            wcat_t = cpool.tile([128, 2048], F16, tag="wcat")
            nc.sync.dma_start(wcat_t[:], wcat_in[:])
            kpcat_t = cpool.tile([128, 48], F32, tag="kpcat")
            nc.sync.dma_start(kpcat_t[:], kpcat_in[:])
            seg_t = cpool.tile([128, CAP], F16, tag="seg")
            nc.sync.dma_start(seg_t[:], seg_in[:])
            ident_t = cpool.tile([128, 128], F16, tag="ident16")
            nc.sync.dma_start(ident_t[:], ident_in[:])
            midx_t = cpool.tile([128, CAP // 16], I16, tag="midx")
            nc.sync.dma_start(midx_t[:], midx_in[:])

            nscidx_t = cpool.tile([128, CAP // 16], I16, tag="nscidx")
            nc.sync.dma_start(nscidx_t[:], nscidx_in[:])
            nreg = nc.gpsimd.to_reg(CAP)
            greg = nc.gpsimd.to_reg(1024)
            done_sems = [nc.alloc_semaphore(f"scat_done{q}") for q in range(4)]
            swctr = [0]

            def swq():
                swctr[0] += 1
                return 0

            import contextlib
            loop_cm = tc.For_i(0, reps, 1) if reps else contextlib.nullcontext()
            with loop_cm:
                # gathers: fT (transpose mode), combined row (feats+coords),
                # query row
                GCH = 1024  # indices per dma_gather call
                NCH = CAP // GCH             # chunks
                BPC = GCH // 128             # blocks per chunk
                qsb = gpool.tile([128, NBLK, 4], F32, tag="qsb")
                nc.sync.dma_start(qsb[:], qsel_in[:].rearrange(
                    "a (b c) -> a b c", c=4))
                ssb_l, ftg_l, kwt_l = [], [], []
                for g in range(NCH):
                    ssb = gpool.tile([128, BPC, 64], F32, tag=f"ssb{g}")
                    if "ssb" in sk:
                        nc.vector.memset(ssb[:], 0.0)
                    else:
                        nc.gpsimd.dma_gather(
                            ssb[:], ftab_in[:].bitcast(F32),
                            midx_t[:, g * GCH // 16:(g + 1) * GCH // 16],
                            GCH, greg, 64, queue_num=swq())
                    ssb_l.append(ssb)
                    # fT via PE transpose, one 128-entry block at a time
                    ftg = gpool.tile([128, GCH], F16, tag=f"ftg{g}")
                    ssb16 = ssb[:].bitcast(F16)      # [128, BPC, 128]
                    for cc in range(BPC):
                        psumT = pstpool.tile([64, 128], F16, tag="pst")
                        nc.tensor.transpose(
                            psumT[:],
                            bass.AP(ssb16.tensor, ssb16.offset + cc * 128,
                                    [ssb16.ap[0], [1, 64]]),
                            ident_t[:])
                        nc.scalar.copy(
                            ap_part(ftg[:], 0, 64, cc * 128, [[1, 128]]),
                            psumT[:])
                    ftg_l.append(ftg)
                    # rel = s - q; kw = relu(1 - sqrt(d2)/sigma)
                    rel = gpool.tile([128, BPC, 3], F32, tag=f"rel{g}")
                    nc.vector.tensor_tensor(
                        out=rel[:],
                        in0=ap_view(ssb[:], 32, [[64, BPC], [1, 3]]),
                        in1=ap_view(qsb[:], g * BPC * 4, [[4, BPC], [1, 3]]),
                        op=mybir.AluOpType.subtract)
                    diff = gpool.tile([128, BPC * 45], F32, tag=f"diff{g}")
                    nc.vector.tensor_tensor(
                        out=ap_view(diff[:], 0, [[45, BPC], [3, P], [1, 3]]),
                        in0=ap_view(rel[:], 0, [[3, BPC], [0, P], [1, 3]]),
                        in1=ap_view(kpcat_t[:], 0, [[0, BPC], [3, P], [1, 3]]),
                        op=mybir.AluOpType.subtract)
                    nc.scalar.activation(diff[:], diff[:],
                                         mybir.ActivationFunctionType.Square,
                                         bias=0.0, scale=1.0)
                    kwt = gpool.tile([128, BPC, 16], F32, tag=f"kw{g}")
                    nc.vector.memset(kwt[:], 0.0)
                    nc.vector.tensor_reduce(
                        out=ap_view(kwt[:], 0, [[16, BPC], [1, P]]),
                        in_=ap_view(diff[:], 0, [[45, BPC], [3, P], [1, 3]]),
                        axis=mybir.AxisListType.X, op=mybir.AluOpType.add)
                    nc.scalar.activation(kwt[:], kwt[:],
                                         mybir.ActivationFunctionType.Sqrt,
                                         bias=kpcat_t[:, 45:46], scale=1.0)
                    nc.scalar.activation(kwt[:], kwt[:],
                                         mybir.ActivationFunctionType.Relu,
                                         bias=1.0, scale=kpcat_t[:, 46:47])
                    kwt_l.append(kwt)

                scat = gpool.tile([128, NBLK, C_OUT], F32, tag="scat")
                lp = nc.allow_low_precision(
                    reason="f16 weighted intermediates; validated vs "
                           "reference at 3e-4 rel err")
                lp.__enter__()
                for c in range(NBLK):
                    wtdm = wpool.tile([128, 2048], F16, tag="wtdm")
                    for hw in range(2):  # p 0:8 | p 8:16 (slot 15 zero-W)
                        psumF = psfpool.tile([128, 1024], F32, tag="psf")
                        for k in range(2):
                            nc.tensor.matmul(
                                psumF[:, k * 512:(k + 1) * 512],
                                ap_part(ftg_l[c // BPC][:], 0, C_IN,
                                        (c % BPC) * 128, [[1, 128]]),
                                ap_part(wcat_t[:], 0, C_IN,
                                        hw * 1024 + k * 512, [[1, 512]]),
                                start=True, stop=True)
                        nc.vector.tensor_tensor(
                            out=ap_view(wtdm[:], hw * 1024,
                                        [[C_OUT, 8], [1, C_OUT]]),
                            in0=ap_view(psumF[:], 0, [[C_OUT, 8], [1, C_OUT]]),
                            in1=ap_view(kwt_l[c // BPC][:],
                                        (c % BPC) * 16 + hw * 8,
                                        [[1, 8], [0, C_OUT]]),
                            op=mybir.AluOpType.mult)
                    tr1 = wpool.tile([128, 1024], F16, tag="tr1")
                    nc.vector.tensor_tensor(
                        out=tr1[:], in0=wtdm[:, 0:1024], in1=wtdm[:, 1024:2048],
                        op=mybir.AluOpType.add)
                    tr2 = wpool.tile([128, 512], F16, tag="tr2")
                    nc.vector.tensor_tensor(
                        out=tr2[:], in0=tr1[:, 0:512], in1=tr1[:, 512:1024],
                        op=mybir.AluOpType.add)
                    tr3 = wpool.tile([128, 256], F16, tag="tr3")
                    nc.vector.tensor_tensor(
                        out=tr3[:], in0=tr2[:, 0:256], in1=tr2[:, 256:512],
                        op=mybir.AluOpType.add)
                    ct = wpool.tile([128, C_OUT], F16, tag="ct")
                    nc.vector.tensor_tensor(
                        out=ct[:], in0=tr3[:, 0:128], in1=tr3[:, 128:256],
                        op=mybir.AluOpType.add)
                    psum2 = ps2pool.tile([128, C_OUT], F32, tag="ps2")
                    nc.tensor.matmul(psum2[:], seg_t[:, c * 128:(c + 1) * 128],
                                     ct[:], start=True, stop=True)
                    nc.scalar.copy(
                        ap_view(scat[:], c * C_OUT, [[1, C_OUT]]), psum2[:])

                lp.__exit__(None, None, None)
                if "scatter" not in sk:
                    qcnt = [0, 0, 0, 0]
                    for g in range(CAP // GCH):
                        q = 0
                        nc.gpsimd.dma_scatter_add(
                            out_t[:],
                            ap_view(scat[:], g * (GCH // 128) * C_OUT,
                                    [[C_OUT, GCH // 128], [1, C_OUT]]),
                            nscidx_t[:, g * GCH // 16:(g + 1) * GCH // 16],
                            GCH, greg, C_OUT,
                            queue_num=q).then_inc(done_sems[q], 16)
                        qcnt[q] += 16
                    for q in range(4):
                        if qcnt[q]:
                            nc.gpsimd.wait_ge(done_sems[q], qcnt[q])
                else:
                    nc.sync.dma_start(out_t[0:128, :],
                                      ap_view(scat[:], 0, [[1, C_OUT]]))
    return nc


def _make_runner_sparse(nc, n_cores):
    bass2jax.install_neuronx_cc_hook()
    from jax.sharding import Mesh, PartitionSpec
    from jax.experimental.shard_map import shard_map

    partition_name = nc.partition_id_tensor.name if nc.partition_id_tensor else None
    in_names, out_names, out_avals = [], [], []
    for alloc in nc.m.functions[0].allocations:
        if not isinstance(alloc, mybir.MemoryLocationSet):
            continue
        name = alloc.memorylocations[0].name
        if alloc.kind == "ExternalInput":
            if name != partition_name:
                in_names.append(name)
        elif alloc.kind == "ExternalOutput":
            shape = tuple(alloc.tensor_shape)
            dtype = mybir.dt.np(alloc.dtype)
            out_names.append(name)
            out_avals.append(jax.core.ShapedArray(shape, dtype))
    n_params = len(in_names)
    n_outs = len(out_avals)
    all_in = in_names + out_names + ([partition_name] if partition_name else [])

    def _body(*args):
        operands = list(args)
        if partition_name is not None:
            operands.append(bass2jax.partition_id_tensor())
        outs = bass2jax._bass_exec_p.bind(
            *operands, out_avals=tuple(out_avals), in_names=tuple(all_in),
            out_names=tuple(out_names), lowering_input_output_aliases=(),
            sim_require_finite=False, sim_require_nnan=False, nc=nc)
        return tuple(outs)

    devices = jax.devices()[:n_cores]
    mesh = Mesh(np.asarray(devices), ("core",))
    in_specs = (PartitionSpec("core"),) * (n_params + n_outs)
    out_specs = (PartitionSpec("core"),) * n_outs
    donate = tuple(range(n_params, n_params + n_outs))
    jit_fn = jax.jit(
        shard_map(_body, mesh=mesh, in_specs=in_specs, out_specs=out_specs,
                  check_rep=False), donate_argnums=donate, keep_unused=True)

    def run(in_maps, out_prefills):
        per_core = [[np.asarray(m[n]) for n in in_names] for m in in_maps]
        args = [np.concatenate([per_core[c][i] for c in range(n_cores)], axis=0)
                for i in range(n_params)]
        args += [np.concatenate([np.asarray(p[n]) for p in out_prefills], axis=0)
                 for n in out_names]
        outs = [np.asarray(o) for o in jit_fn(*args)]
        return [{n: outs[i].reshape(n_cores, *out_avals[i].shape)[c]
                 for i, n in enumerate(out_names)}
                for c in range(n_cores)], jit_fn, args

    return run


def _get_runner_sparse():
    if "sparse" not in _BUILT:
        nc = build_sparse()
        _BUILT["sparse"] = _make_runner_sparse(nc, N_CORES)
    return _BUILT["sparse"]


def _wrap16(vals, pad_val, dtype=np.int16):
    """List -> [128, CAP//16] wrapped (entry j at [j%16, j//16]), replicated
    across the 8 gpsimd cores."""
    buf = np.full(CAP, pad_val, dtype)
    buf[:len(vals)] = vals
    w = buf.reshape(CAP // 16, 16).T          # [16, CAP//16]
    return np.ascontiguousarray(np.tile(w, (8, 1)))


def _host_prep_sparse(qp, sp, sf, ni, w, bias_v, kpv):
    """Returns (in_maps, out_prefills) or None if candidates exceed CAP."""
    # conservative candidate radius: a hit needs |s - q| < sigma + max|kp|
    rmax = SIGMA + float(np.sqrt((kpv * kpv).sum(axis=1)).max())
    t_cell = (rmax * GRID + math.sqrt(3.0)) ** 2
    scell = np.clip((sp * GRID).astype(np.int32), 0, GRID - 1)
    qcell = np.clip((qp * GRID).astype(np.int32), 0, GRID - 1)

    wcat = np.zeros((128, 2048), np.float16)
    wcat[:C_IN, :P * C_OUT] = (np.transpose(w, (1, 0, 2)) / 16.0
                               ).reshape(C_IN, -1)
    wcat[C_IN:] = wcat[:C_IN]
    kpcat = np.zeros((128, 48), np.float32)
    kpcat[:, :45] = kpv.reshape(1, 45)
    kpcat[:, 45] = 1e-10
    kpcat[:, 46] = -1.0 / SIGMA

    in_maps, out_prefills = [], []
    for c in range(N_CORES):
        b, half = divmod(c, 2)
        n0 = half * NQ_CORE
        nib = ni[b, n0:n0 + NQ_CORE]
        dc = scell[b][nib] - qcell[b, n0:n0 + NQ_CORE, None, :]
        d2 = (dc.astype(np.int64) ** 2).sum(axis=2)
        nn, kk = np.nonzero(d2 <= t_cell)      # sorted by n (row-major)
        mm = nib[nn, kk]

        # pack into 128-entry blocks; no query group spans a block boundary
        m_list = np.zeros(CAP, np.int16)
        n_list = np.zeros(CAP, np.int16)
        seg = np.zeros((128, CAP), np.float16)
        sc_idx = np.full(CAP, TRASH, np.int16)
        uniq, counts = np.unique(nn, return_counts=True)
        t = 0            # global entry cursor
        gi = 0           # group cursor
        ok = True
        off = 0          # start of each group's pairs in nn/kk
        for g in range(len(uniq)):
            cnt = counts[g]
            blk, pos = divmod(t, 128)
            if pos + cnt > 128:                # pad to next block
                t = (blk + 1) * 128
                blk, pos = blk + 1, 0
            if t + cnt > CAP:
                ok = False
                break
            d = 127                            # d-slot for this group
            # d slots allocated in order of first use within the block
            # (track per-block next free slot)
            m_list[t:t + cnt] = mm[off:off + cnt]
            n_list[t:t + cnt] = uniq[g]
            t += cnt
            off += cnt
        if not ok:
            return None
        # second pass: assign d-slots and seg/sc_idx now that layout is fixed
        seg[:] = 0
        sc_idx[:] = TRASH
        blk_next = np.zeros(NBLK, np.int32)
        t = 0
        off = 0
        for g in range(len(uniq)):
            cnt = counts[g]
            blk, pos = divmod(t, 128)
            if pos + cnt > 128:
                t = (blk + 1) * 128
                blk, pos = blk + 1, 0
            d = blk_next[blk]
            blk_next[blk] += 1
            seg[pos:pos + cnt, blk * 128 + d] = 1.0
            sc_idx[blk * 128 + d] = uniq[g]
            t += cnt
            off += cnt
        # pad entries (between groups / tail): m=0, n=0 gathers; their seg
        # column stays 0 -> contribute nothing; unused d-slots scatter to
        # TRASH row.

        ftab = np.zeros((M, 128), np.float16)
        ftab[:, :C_IN] = sf[b]
        ftab.view(np.float32)[:, 32:35] = sp[b]
        qsel = np.zeros((CAP, 4), np.float32)
        qsel[:, :3] = qp[b, n0 + n_list.astype(np.int64)]
        qsel = np.ascontiguousarray(
            qsel.reshape(NBLK, 128, 4).transpose(1, 0, 2)).reshape(128, -1)
        in_maps.append({
            "ftab": ftab, "qsel": qsel, "wcat": wcat, "kpcat": kpcat,
            "seg": seg, "ident16": np.eye(128, dtype=np.float16),
            "midx": _wrap16(m_list, 0),
            "nscidx": _wrap16(sc_idx, TRASH),
        })
        out_prefills.append({
            "out": np.tile(bias_v.reshape(1, C_OUT),
                           (NQ_CORE + 1, 1)).astype(np.float32)})
    return in_maps, out_prefills


def _kernel_dense(qp_raw, sp_raw, sf_raw, ni_raw, w_raw, bias_raw, kp_raw):
    kp = np.asarray(kp_raw, np.float32)
    run = _get_runner(kp)
    in_maps = _host_prep(qp_raw, sp_raw, sf_raw, ni_raw, w_raw, bias_raw,
                         kp_raw)
    results, _, _ = run(in_maps)
    out = np.zeros((B, N, C_OUT), np.float32)
    for c in range(N_CORES):
        b, half = divmod(c, 2)
        n0 = half * NQ_CORE
        out[b, n0:n0 + NQ_CORE, :] = results[c]["out"]
    return out


def kernel(query_points, support_points, support_features, neighbor_indices,
           weights, bias, kernel_points):
    qp = np.asarray(query_points, np.float32)
    sp = np.asarray(support_points, np.float32)
    sf = np.asarray(support_features, np.float32)
    ni = np.clip(np.asarray(neighbor_indices), 0, M - 1).astype(np.int32)
    w = np.asarray(weights, np.float32)
    bias_v = np.asarray(bias, np.float32)
    kpv = np.asarray(kernel_points, np.float32)

    prep = _host_prep_sparse(qp, sp, sf, ni, w, bias_v, kpv)
    if prep is None:
        return _kernel_dense(query_points, support_points, support_features,
                             neighbor_indices, weights, bias, kernel_points)
    in_maps, out_prefills = prep
    run = _get_runner_sparse()
    results, _, _ = run(in_maps, out_prefills)
    out = np.zeros((B, N, C_OUT), np.float32)
    for c in range(N_CORES):
        b, half = divmod(c, 2)
        n0 = half * NQ_CORE
        out[b, n0:n0 + NQ_CORE, :] = results[c]["out"][:NQ_CORE]

    # exact neighbor-count correction (reference divides by the number of
    # neighbors with nonzero features, clipped to >= 1; the device divides
    # by K=16). For randn features every row is nonzero, so cnt == 16 and
    # this is a no-op; handle degenerate inputs on host for full fidelity.
    row_nz = np.abs(sf).sum(axis=2) > 0          # [B, M]
    if not row_nz.all():
        z = row_nz.astype(np.float32)
        cnt = np.clip(
            z[np.arange(B)[:, None, None], ni].sum(axis=2), 1.0, None)
        out = (out - bias_v) * (16.0 / cnt)[..., None] + bias_v
    return out



# revision 38
# speedup vs baseline: 1.2312x; 1.1969x over previous
"""KPConv (nn_KPConvFPN) Trainium2 Bass kernel — sparse candidate-pair design.

kw = relu(1 - |s[m] - q[n] - kp_p|/sigma) is ~97.6% zero for these inputs
(points uniform in [0,1]^3, sigma + max|kp| = 0.0825). The host finds a
conservative SUPERSET of candidate (query, neighbor) pairs by integer cell
binning (GRID=128; any pair within reach of any kernel point is provably
included; no float math decides output values on the host). Per core
(batch b=c//2, query half c%2):

Device pipeline (CAP=5120 candidate pairs, 40 blocks of 128):
  1. Per 1024-pair chunk: SWDGE dma_gather of combined 256B rows
     [64 f16 feats | s-coords f32] from ftab; PE-transpose feats -> fT;
     DVE/ACT compute kw[t, p] for all 15 kernel points.
     (query coords arrive pre-gathered from host as qsel, like the dense
     kernel's qrep.)
  2. Per 128-pair block: 4 PE matmuls fW = fT @ [W_0|..|W_15]/16 (f16,
     2048 psum cols); DVE multiply by kw broadcast over C_out; binary-tree
     add over the 16 p-slots -> ct[t, 128].
  3. Segment matmul psum2[d, o] = seg[t, d]^T @ ct (host-built 0/1 seg
     matrix; groups pairs of the same query; pads/unused -> trash slot),
     so every output row is scattered EXACTLY once (dma_scatter_add loses
     updates on duplicate rows -- measured).
  4. dma_scatter_add rows into the bias-prefilled donated output buffer
     (row 8192 = trash row for pad slots).

Falls back to the dense kernel (build_bass below) when candidates exceed
CAP. The reference divides by the count of neighbors with nonzero
features; for randn features that is always K=16 (folded into W/16); the
degenerate case is corrected exactly on the host.
"""
import json
import math
import os

SKIP = set()

import numpy as np
import jax

import concourse.bass as bass
import concourse.mybir as mybir
from concourse.tile import TileContext
from concourse import library_config
from concourse import bass2jax

F32 = mybir.dt.float32
F16 = mybir.dt.float16
I16 = mybir.dt.int16

B, N, M, K = 4, 16384, 16384, 16
C_IN, C_OUT, P = 64, 128, 15
SIGMA = 0.03
N_CORES = 8
NQ_CORE = N // 2            # 8192 queries per core
NK_CORE = NQ_CORE * K       # 131072 gathered rows per core
ST_Q = 512                  # queries per supertile
N_ST = NQ_CORE // ST_Q      # 16
KW_ST = 2                   # supertiles per kw group
G_ST = ST_Q * K // 128      # 64 g-cols per supertile
ROW16 = 128                 # fp16 units per table row (256B)

# ---------------------------------------------------------------------------
# walrus workaround: this nix walrus build supports ONE sync-wait per
# instruction; split extra waits onto NoOps inserted before the offender
# (same-engine program order preserves semantics). Also run
# codegen_inst_isa_subclasses (Bacc does; raw Bass doesn't) so extended
# instructions get their ISA bytes.
_orig_to_json_bytes = bass.Bass.to_json_bytes


def _fix_block(bb, ctr):
    insts = bb.get("instructions")
    if not isinstance(insts, list):
        return
    new = []
    for inst in insts:
        si = inst.get("sync_info")
        ow = si.get("on_wait") if isinstance(si, dict) else None
        if ow and len(ow) > 1:
            for w in ow[:-1]:
                ctr[0] += 1
                nop = {"engine": inst["engine"], "ins": [], "outs": [],
                       "name": f"I-wsplit-{ctr[0]}", "opcode": "NoOp",
                       "sync_info": {"on_update": [], "on_wait": [w]},
                       "text_hint": "wsplit"}
                if "debug" in inst:
                    nop["debug"] = inst["debug"]
                new.append(nop)
            si["on_wait"] = [ow[-1]]
        new.append(inst)
    bb["instructions"] = new


def _walk(o, ctr):
    if isinstance(o, dict):
        if isinstance(o.get("instructions"), list):
            _fix_block(o, ctr)
        for v in o.values():
            _walk(v, ctr)
    elif isinstance(o, list):
        for v in o:
            _walk(v, ctr)


def _to_json_bytes_split(self):
    mybir.codegen_inst_isa_subclasses(self)
    raw = _orig_to_json_bytes(self)
    d = json.loads(raw)
    ctr = [0]
    _walk(d, ctr)
    return json.dumps(d).encode()


bass.Bass.to_json_bytes = _to_json_bytes_split


def ap_view(t_ap, extra_offset, dims):
    """AP over tile t_ap with explicit free dims [[step, count], ...]
    (steps in elements); partition dim is taken from the tile."""
    return bass.AP(t_ap.tensor, t_ap.offset + extra_offset,
                   [t_ap.ap[0]] + list(dims))


def ap_part(t_ap, pstart, pcount, extra_offset, dims):
    pstep = t_ap.ap[0][0]
    return bass.AP(t_ap.tensor, t_ap.offset + pstart * pstep + extra_offset,
                   [[pstep, pcount]] + list(dims))


def build_bass(kp, reps=0, skip=()):
    global SKIP
    SKIP = set(skip)
    """kp: (15, 3) float32 numpy kernel points (runtime values baked)."""
    kpsq = (kp * kp).sum(axis=1)  # |kp_p|^2
    nc = bass.Bass(dynamic_dma_scratch_size=32768, num_swdge_queues=4)

    feats_in = nc.dram_tensor("sfeat", [M, C_IN], F32, kind="ExternalInput")
    pts_in = nc.dram_tensor("spts", [M, 3], F32, kind="ExternalInput")
    qrep_in = nc.dram_tensor("qrep", [128, NK_CORE // 128, 3], F32,
                             kind="ExternalInput")
    idx_in = nc.dram_tensor("idx", [128, NK_CORE // 16], I16,
                            kind="ExternalInput")
    w_in = nc.dram_tensor("w", [P, C_IN, C_OUT], F32, kind="ExternalInput")
    bias_in = nc.dram_tensor("bias", [C_OUT, 1], F32, kind="ExternalInput")
    mask120_in = nc.dram_tensor("mask120", [128, 120], F32, kind="ExternalInput")
    mask16_in = nc.dram_tensor("mask16", [128, 8], F32, kind="ExternalInput")
    ident_in = nc.dram_tensor("ident", [128, 128], F32, kind="ExternalInput")
    ones1_in = nc.dram_tensor("ones1", [1, 128], F32, kind="ExternalInput")
    kpb_in = nc.dram_tensor("kpb", [128, 48], F32, kind="ExternalInput")
    onesc_in = nc.dram_tensor("onesc", [128, 1], F32, kind="ExternalInput")
    out_t = nc.dram_tensor("out", [NQ_CORE, C_OUT], F32, kind="ExternalOutput")
    table = nc.dram_tensor("table", [M, ROW16], F16, kind="Internal")

    # library load as raw preamble (before Tile scheduling) so it is
    # guaranteed to precede every dma_gather on the Pool engine.
    nc.gpsimd.load_library(library_config.mlp)

    with TileContext(nc) as tc:
        with tc.tile_pool(name="const", bufs=1) as cpool, \
             tc.tile_pool(name="build", bufs=1) as bpool, \
             tc.tile_pool(name="gath", bufs=2) as gpool, \
             tc.tile_pool(name="kwp", bufs=2) as kwpool, \
             tc.tile_pool(name="kbd", bufs=1) as kbpool, \
             tc.tile_pool(name="wt", bufs=1) as wtpool, \
             tc.tile_pool(name="sm", bufs=3) as smpool, \
             tc.tile_pool(name="fin", bufs=2) as fpool, \
             tc.tile_pool(name="ps1", bufs=2, space="PSUM") as ps1pool, \
             tc.tile_pool(name="ps2", bufs=2, space="PSUM") as ps2pool, \
             tc.tile_pool(name="ps3", bufs=1, space="PSUM") as ps3pool:

            # ---- constants ----
            wp_t = cpool.tile([C_IN, P * C_OUT], F32, tag="wp")
            nc.sync.dma_start(
                wp_t[:].rearrange("c (p o) -> c p o", p=P),
                w_in[:].rearrange("p c o -> c p o"))
            bias_t = cpool.tile([C_OUT, 1], F32, tag="bias")
            nc.sync.dma_start(bias_t[:], bias_in[:])
            mask120_t = cpool.tile([128, 120], F32, tag="m120")
            nc.sync.dma_start(mask120_t[:], mask120_in[:])
            mask16_t = cpool.tile([128, 8], F32, tag="m16")
            nc.sync.dma_start(mask16_t[:], mask16_in[:])
            ident_t = cpool.tile([128, 128], F32, tag="ident")
            nc.sync.dma_start(ident_t[:], ident_in[:])
            ones1_t = cpool.tile([1, 128], F32, tag="ones1")
            nc.sync.dma_start(ones1_t[:], ones1_in[:])
            kpb_t = cpool.tile([128, 48], F32, tag="kpb")
            nc.sync.dma_start(kpb_t[:], kpb_in[:])
            onesc_t = cpool.tile([128, 1], F32, tag="onesc")
            nc.sync.dma_start(onesc_t[:], onesc_in[:])
            nidx_reg = nc.gpsimd.to_reg(1024)

            # ---- 1. combined table build (8 chunks x 2048 rows) ----
            import contextlib
            loop_cm = tc.For_i(0, reps, 1) if reps else contextlib.nullcontext()
            with loop_cm:
                _table_build(nc, tc, bpool, feats_in, pts_in, table)
                _main_pipeline(nc, tc, gpool, kwpool, kbpool, wtpool, smpool,
                               fpool, ps1pool, ps2pool, ps3pool, kp,
                               qrep_in, idx_in, out_t, table, wp_t, bias_t,
                               mask120_t, mask16_t, ident_t, ones1_t, kpb_t,
                               onesc_t, nidx_reg)
    return nc


def _table_build(nc, tc, bpool, feats_in, pts_in, table):
            for ch in range(8):
                m0 = ch * 2048
                fsb = bpool.tile([128, 16, C_IN], F32, tag="fsb")
                nc.sync.dma_start(
                    fsb[:],
                    feats_in[m0:m0 + 2048, :].rearrange(
                        "(a p) c -> p a c", p=128))
                psb = bpool.tile([128, 16, 3], F32, tag="psb")
                nc.sync.dma_start(
                    psb[:],
                    pts_in[m0:m0 + 2048, :].rearrange(
                        "(a p) c -> p a c", p=128))
                st16 = bpool.tile([128, 16, ROW16], F16, tag="st16")
                nc.vector.tensor_copy(st16[:, :, 0:C_IN], fsb[:])
                stv32 = st16[:].bitcast(F32)  # [128, 16, 64] f32 view
                # aux: sx sy sz at f32-cols 32..34
                nc.vector.tensor_copy(
                    bass.AP(stv32.tensor, stv32.offset + 32,
                            [stv32.ap[0], [64, 16], [1, 3]]),
                    psb[:])
                # |s|^2 at f32-col 35
                psq = bpool.tile([128, 16, 3], F32, tag="psq")
                nc.vector.tensor_tensor(out=psq[:], in0=psb[:], in1=psb[:],
                                        op=mybir.AluOpType.mult)
                nc.vector.tensor_reduce(
                    out=bass.AP(stv32.tensor, stv32.offset + 35,
                                [stv32.ap[0], [64, 16], [1, 1]]),
                    in_=psq[:], axis=mybir.AxisListType.X,
                    op=mybir.AluOpType.add)
                # z at f32-col 36: (sum_c |f|) > 0
                zred = bpool.tile([128, 16, 1], F32, tag="zred")
                nc.vector.tensor_reduce(out=zred[:], in_=fsb[:],
                                        axis=mybir.AxisListType.X,
                                        op=mybir.AluOpType.add,
                                        apply_absolute_value=True)
                nc.vector.tensor_scalar(
                    out=bass.AP(stv32.tensor, stv32.offset + 36,
                                [stv32.ap[0], [64, 16], [1, 1]]),
                    in0=zred[:], scalar1=0.0, scalar2=None,
                    op0=mybir.AluOpType.is_gt)
                nc.sync.dma_start(
                    table[m0:m0 + 2048, :].rearrange("(a p) c -> p a c",
                                                     p=128),
                    st16[:])


def _main_pipeline(nc, tc, gpool, kwpool, kbpool, wtpool, smpool, fpool,
                   ps1pool, ps2pool, ps3pool, kp, qrep_in, idx_in, out_t,
                   table, wp_t, bias_t, mask120_t, mask16_t, ident_t,
                   ones1_t, kpb_t, onesc_t, nidx_reg):
            for kg in range(N_ST // KW_ST):  # kw group of 2 supertiles
                GQ = KW_ST * ST_Q            # 1024 queries
                GG = KW_ST * G_ST            # 128 g-cols
                gt = gpool.tile([128, GG, ROW16], F16, tag="gath")
                gt32 = gt[:].bitcast(F32)  # [128, GG, 64] f32 view
                # gathers: 16 chunks of 1024 idx
                if "gather" in SKIP:
                    nc.vector.memset(gt[:], 0.0)
                for g in range(GG // 8):
                    if "gather" in SKIP:
                        break
                    idxsl = smpool.tile([128, 64], I16, tag="idxsl")
                    nc.sync.dma_start(
                        idxsl[:],
                        idx_in[:, (kg * 16 + g) * 64:(kg * 16 + g) * 64 + 64])
                    nc.gpsimd.dma_gather(
                        gt[:, g * 8:(g + 1) * 8, :], table[:], idxsl[:],
                        1024, nidx_reg, ROW16, queue_num=g % 4)
                # qrep slice
                qr = smpool.tile([128, GG, 3], F32, tag="qr")
                nc.sync.dma_start(qr[:], qrep_in[:, kg * GG:(kg + 1) * GG, :])
                # rel = s - q
                rel = smpool.tile([128, GG, 3], F32, tag="rel")
                nc.vector.tensor_tensor(
                    out=rel[:],
                    in0=ap_view(gt32, 32, [[64, GG], [1, 3]]),
                    in1=qr[:], op=mybir.AluOpType.subtract)
                # d2[p] = sum_dim (rel_dim - kp[p,dim])^2  (ACT squares, DVE adds)
                kwt = kwpool.tile([128, GG, P], F32, tag="kw")
                sq0 = smpool.tile([128, GG], F32, tag="sq0")
                sq1 = smpool.tile([128, GG], F32, tag="sq1")
                if "kw" in SKIP:
                    nc.vector.memset(kwt[:], 0.0)
                for p in range(P if "kw" not in SKIP else 0):
                    d2dst = ap_view(kwt[:], p, [[P, GG], [1, 1]])
                    nc.scalar.activation(
                        sq0[:], ap_view(rel[:], 0, [[3, GG], [1, 1]]),
                        mybir.ActivationFunctionType.Square,
                        bias=kpb_t[:, 3 * p:3 * p + 1], scale=1.0)
                    nc.scalar.activation(
                        sq1[:], ap_view(rel[:], 1, [[3, GG], [1, 1]]),
                        mybir.ActivationFunctionType.Square,
                        bias=kpb_t[:, 3 * p + 1:3 * p + 2], scale=1.0)
                    nc.vector.tensor_tensor(out=sq0[:], in0=sq0[:],
                                            in1=sq1[:],
                                            op=mybir.AluOpType.add)
                    nc.scalar.activation(
                        sq1[:], ap_view(rel[:], 2, [[3, GG], [1, 1]]),
                        mybir.ActivationFunctionType.Square,
                        bias=kpb_t[:, 3 * p + 2:3 * p + 3], scale=1.0)
                    nc.vector.tensor_tensor(out=d2dst, in0=sq0[:],
                                            in1=sq1[:],
                                            op=mybir.AluOpType.add)
                # kw = relu(1 - sqrt(d2 + 1e-10)/sigma), in place
                if "kw" in SKIP:
                    pass
                else:
                    nc.scalar.activation(kwt[:], kwt[:],
                                     mybir.ActivationFunctionType.Sqrt,
                                     bias=kpb_t[:, 45:46], scale=1.0)
                if "kw" not in SKIP:
                    nc.scalar.activation(kwt[:], kwt[:],
                                     mybir.ActivationFunctionType.Relu,
                                     bias=1.0, scale=kpb_t[:, 46:47])

                for sti in range(KW_ST):
                    st = kg * KW_ST + sti
                    # kwbd (2 half-ST TT ops): [128, (bl32, q8, p15)] fp16
                    kbd = kbpool.tile([128, 3840], F16, tag="kbd")
                    kbd2 = kbpool.tile([128, 3840], F16, tag="kbd2")
                    if "kwbd" in SKIP:
                        nc.vector.memset(kbd[:], 0.0)
                        nc.vector.memset(kbd2[:], 0.0)
                    for hf, kb in ((0, kbd), (1, kbd2)) if "kwbd" not in SKIP else ():
                        bl0 = sti * G_ST + hf * 32
                        nc.vector.tensor_tensor(
                            out=ap_view(kb[:], 0,
                                        [[120, 32], [15, 8], [1, 15]]),
                            in0=ap_view(kwt[:], bl0 * P,
                                        [[P, 32], [0, 8], [1, P]]),
                            in1=ap_view(mask120_t[:], 0,
                                        [[0, 32], [15, 8], [1, 15]]),
                            op=mybir.AluOpType.mult)
                    # einsum1: 64 blocks
                    wtt = wtpool.tile([64, 7680], F32, tag="wt")
                    if "e1" in SKIP:
                        nc.vector.memset(wtt[:], 0.0)
                    for bg in range(16 if "e1" not in SKIP else 0):  # bank groups of 4 blocks (32 q)
                        pse1 = ps1pool.tile([64, 480], F32, tag="pse1")
                        for j in range(4):
                            bl = bg * 4 + j          # block in supertile
                            blg = sti * G_ST + bl    # g-col in group tile
                            kb = kbd if bl < 32 else kbd2
                            kbl = bl % 32
                            nc.tensor.matmul(
                                pse1[:, j * 120:(j + 1) * 120],
                                ap_view(gt[:], blg * ROW16, [[1, C_IN]]),
                                ap_view(kb[:], kbl * 120, [[1, 120]]),
                                start=True, stop=True)
                        # evict (split DVE/ACT)
                        nc.vector.tensor_copy(
                            wtt[:, bg * 480:bg * 480 + 240],
                            pse1[:, 0:240])
                        nc.scalar.copy(
                            wtt[:, bg * 480 + 240:bg * 480 + 480],
                            pse1[:, 240:480])
                    # count row: zbd = z * mask16 -> ones-row matmul
                    zbd = smpool.tile([128, 512], F32, tag="zbd")
                    nc.vector.tensor_tensor(
                        out=zbd[:].rearrange("a (g j q) -> a g j q",
                                             g=16, j=4),
                        in0=ap_view(gt32, (sti * G_ST) * 64 + 36,
                                    [[256, 16], [64, 4], [0, 8]]),
                        in1=ap_view(mask16_t[:], 0,
                                    [[0, 16], [0, 4], [1, 8]]),
                        op=mybir.AluOpType.mult)
                    pscnt = ps3pool.tile([1, 512], F32, tag="pscnt")
                    nc.tensor.matmul(pscnt[:], onesc_t[:], zbd[:],
                                     start=True, stop=True)
                    cntinv = smpool.tile([1, 512], F32, tag="cntinv")
                    nc.vector.tensor_scalar(out=cntinv[:], in0=pscnt[:],
                                            scalar1=1.0, scalar2=None,
                                            op0=mybir.AluOpType.max)
                    nc.vector.reciprocal(out=cntinv[:], in_=cntinv[:])
                    psrep = ps3pool.tile([128, 512], F32, tag="psrep")
                    nc.tensor.matmul(psrep[:], ones1_t[:], cntinv[:],
                                     start=True, stop=True)
                    # note: psrep = cntinv^T replicated? see host mapping
                    cntrep = smpool.tile([128, 512], F32, tag="cntrep")
                    nc.vector.tensor_copy(cntrep[:], psrep[:])

                    # einsum2: out[o, s] accumulated over p
                    pse2 = ps2pool.tile([128, 512], F32, tag="pse2")
                    for p in range(P if "e2" not in SKIP else 1):
                        nc.tensor.matmul(
                            pse2[:],
                            ap_view(wp_t[:], p * C_OUT, [[1, C_OUT]]),
                            ap_view(wtt[:], p,
                                    [[480, 16], [120, 4], [15, 8]]),
                            start=(p == 0), stop=True)
                    # divide by count, add bias
                    e2sb = fpool.tile([128, 512], F32, tag="e2sb")
                    nc.vector.tensor_tensor(out=e2sb[:], in0=pse2[:],
                                            in1=cntrep[:],
                                            op=mybir.AluOpType.mult)
                    nc.vector.tensor_scalar(out=e2sb[:], in0=e2sb[:],
                                            scalar1=bias_t[:],
                                            scalar2=None,
                                            op0=mybir.AluOpType.add)
                    # transpose 4x128 cols and store
                    for t4 in range(4):
                        pstr = ps3pool.tile([128, 128], F32, tag="pstr")
                        nc.tensor.transpose(
                            pstr[:], e2sb[:, t4 * 128:(t4 + 1) * 128],
                            ident_t[:])
                        trsb = fpool.tile([128, 128], F32, tag="trsb")
                        nc.scalar.copy(trsb[:], pstr[:])
                        # e2 cols are n-linear: plain contiguous store
                        n0 = st * 512 + t4 * 128
                        nc.sync.dma_start(out_t[n0:n0 + 128, :], trsb[:])


def _make_runner(nc, n_cores):
    bass2jax.install_neuronx_cc_hook()
    from jax.sharding import Mesh, PartitionSpec
    from jax.experimental.shard_map import shard_map

    partition_name = nc.partition_id_tensor.name if nc.partition_id_tensor else None
    in_names, out_names, out_avals, zero_outs = [], [], [], []
    for alloc in nc.m.functions[0].allocations:
        if not isinstance(alloc, mybir.MemoryLocationSet):
            continue
        name = alloc.memorylocations[0].name
        if alloc.kind == "ExternalInput":
            if name != partition_name:
                in_names.append(name)
        elif alloc.kind == "ExternalOutput":
            shape = tuple(alloc.tensor_shape)
            dtype = mybir.dt.np(alloc.dtype)
            out_names.append(name)
            out_avals.append(jax.core.ShapedArray(shape, dtype))
            zero_outs.append(np.zeros(shape, dtype))
    n_params = len(in_names)
    n_outs = len(out_avals)
    all_in = in_names + out_names + ([partition_name] if partition_name else [])

    def _body(*args):
        operands = list(args)
        if partition_name is not None:
            operands.append(bass2jax.partition_id_tensor())
        outs = bass2jax._bass_exec_p.bind(
            *operands, out_avals=tuple(out_avals), in_names=tuple(all_in),
            out_names=tuple(out_names), lowering_input_output_aliases=(),
            sim_require_finite=False, sim_require_nnan=False, nc=nc)
        return tuple(outs)

    devices = jax.devices()[:n_cores]
    mesh = Mesh(np.asarray(devices), ("core",))
    in_specs = (PartitionSpec("core"),) * (n_params + n_outs)
    out_specs = (PartitionSpec("core"),) * n_outs
    jit_fn = jax.jit(
        shard_map(_body, mesh=mesh, in_specs=in_specs, out_specs=out_specs,
                  check_rep=False), keep_unused=True)

    def run(in_maps):
        per_core = [[np.asarray(m[n]) for n in in_names] for m in in_maps]
        args = [np.concatenate([per_core[c][i] for c in range(n_cores)], axis=0)
                for i in range(n_params)]
        args += [np.zeros((n_cores * z.shape[0], *z.shape[1:]), z.dtype)
                 for z in zero_outs]
        outs = [np.asarray(o) for o in jit_fn(*args)]
        return [{n: outs[i].reshape(n_cores, *out_avals[i].shape)[c]
                 for i, n in enumerate(out_names)}
                for c in range(n_cores)], jit_fn, args

    return run


_BUILT = {}


def _get_runner(kp):
    key = kp.tobytes()
    if key not in _BUILT:
        nc = build_bass(kp)
        _BUILT[key] = _make_runner(nc, N_CORES)
    return _BUILT[key]


def _host_prep(query_points, support_points, support_features,
               neighbor_indices, weights, bias, kernel_points):
    qp = np.asarray(query_points, np.float32)
    sp = np.asarray(support_points, np.float32)
    sf = np.asarray(support_features, np.float32)
    ni = np.asarray(neighbor_indices)
    ni = np.clip(ni, 0, M - 1).astype(np.int16)
    w = np.ascontiguousarray(np.asarray(weights, np.float32))
    bias = np.asarray(bias, np.float32).reshape(C_OUT, 1)

    mask120 = np.zeros((128, 120), np.float32)
    for q in range(8):
        mask120[q * 16:(q + 1) * 16, q * 15:(q + 1) * 15] = 1.0
    mask16 = np.zeros((128, 8), np.float32)
    for q in range(8):
        mask16[q * 16:(q + 1) * 16, q] = 1.0
    ident = np.eye(128, dtype=np.float32)
    ones1 = np.ones((1, 128), np.float32)
    kpv = np.asarray(kernel_points, np.float32)
    kpb = np.zeros((128, 48), np.float32)
    for p in range(P):
        for d in range(3):
            kpb[:, 3 * p + d] = -kpv[p, d]
    kpb[:, 45] = 1e-10
    kpb[:, 46] = -1.0 / SIGMA

    in_maps = []
    for c in range(N_CORES):
        b, half = divmod(c, 2)
        n0 = half * NQ_CORE
        idx = ni[b, n0:n0 + NQ_CORE, :].reshape(NK_CORE)
        # chunk order: idx j in chunk -> partition j%16 (k), col j//16;
        # stream order is already (query, k) = natural
        idx_l = idx.reshape(NK_CORE // 16, 16).T          # [16, NK/16]
        idx_l = np.tile(idx_l, (8, 1))                    # [128, NK/16]
        qrep = np.repeat(qp[b, n0:n0 + NQ_CORE, :], K, axis=0)  # [NK, 3]
        qrep = qrep.reshape(NK_CORE // 128, 128, 3).transpose(1, 0, 2)
        qrep = np.ascontiguousarray(qrep)
        in_maps.append({
            "sfeat": sf[b], "spts": sp[b], "qrep": qrep,
            "idx": np.ascontiguousarray(idx_l),
            "w": w, "bias": bias, "mask120": mask120, "mask16": mask16,
            "ident": ident, "ones1": ones1, "kpb": kpb,
            "onesc": np.ones((128, 1), np.float32),
        })
    return in_maps


# ===========================================================================
# Sparse path: kw = relu(1 - d/sigma) is ~99.99% zero for these inputs
# (support/query points uniform in [0,1]^3, sigma=0.03). Host finds a
# conservative SUPERSET of candidate (query, neighbor) pairs by integer
# cell binning (no float math decides values, only candidate pruning; any
# pair within reach of any kernel point is provably included). The device
# gathers those pairs' coords + features, computes exact kw and the two
# einsums for just those pairs, and scatter-adds into the bias-prefilled
# output. Falls back to the dense kernel when candidates exceed CAP.
# ===========================================================================
CAP = 5120          # static per-core candidate-pair capacity (40 blocks)
GRID = 128          # cells per axis for host binning
NBLK = CAP // 128
TRASH = NQ_CORE     # out_t row 8192 = trash for pad/unused slots


def build_sparse(reps=0, skip=()):
    sk = set(skip)
    nc = bass.Bass(dynamic_dma_scratch_size=32768, num_swdge_queues=4)

    ftab_in = nc.dram_tensor("ftab", [M, 128], F16, kind="ExternalInput")
    qsel_in = nc.dram_tensor("qsel", [128, (CAP // 128) * 4], F32,
                             kind="ExternalInput")
    wcat_in = nc.dram_tensor("wcat", [128, 2048], F16, kind="ExternalInput")
    kpcat_in = nc.dram_tensor("kpcat", [128, 48], F32, kind="ExternalInput")
    seg_in = nc.dram_tensor("seg", [128, CAP], F16, kind="ExternalInput")
    ident_in = nc.dram_tensor("ident16", [128, 128], F16, kind="ExternalInput")
    midx_in = nc.dram_tensor("midx", [128, CAP // 16], I16, kind="ExternalInput")
    nscidx_in = nc.dram_tensor("nscidx", [128, CAP // 16], I16, kind="ExternalInput")
    out_t = nc.dram_tensor("out", [NQ_CORE + 1, C_OUT], F32, kind="ExternalOutput")

    nc.gpsimd.load_library(library_config.mlp)

    with TileContext(nc) as tc:
        with tc.tile_pool(name="const", bufs=1) as cpool, \
             tc.tile_pool(name="gath", bufs=1) as gpool, \
             tc.tile_pool(name="work", bufs=2) as wpool, \
             tc.tile_pool(name="psf", bufs=2, space="PSUM") as psfpool, \
             tc.tile_pool(name="pst", bufs=2, space="PSUM") as pstpool, \
             tc.tile_pool(name="ps2", bufs=2, space="PSUM") as ps2pool:
            wcat_t = cpool.tile([128, 2048], F16, tag="wcat")
            nc.sync.dma_start(wcat_t[:], wcat_in[:])
            kpcat_t = cpool.tile([128, 48], F32, tag="kpcat")
            nc.sync.dma_start(kpcat_t[:], kpcat_in[:])
            seg_t = cpool.tile([128, CAP], F16, tag="seg")
            nc.sync.dma_start(seg_t[:], seg_in[:])
            ident_t = cpool.tile([128, 128], F16, tag="ident16")
            nc.sync.dma_start(ident_t[:], ident_in[:])
            midx_t = cpool.tile([128, CAP // 16], I16, tag="midx")
            nc.sync.dma_start(midx_t[:], midx_in[:])

            nscidx_t = cpool.tile([128, CAP // 16], I16, tag="nscidx")
            nc.sync.dma_start(nscidx_t[:], nscidx_in[:])
            nreg = nc.gpsimd.to_reg(CAP)
            greg = nc.gpsimd.to_reg(1024)
            done_sems = [nc.alloc_semaphore(f"scat_done{q}") for q in range(4)]
            swctr = [0]

            def swq():
                swctr[0] += 1
                return 0

            import contextlib
            loop_cm = tc.For_i(0, reps, 1) if reps else contextlib.nullcontext()
            with loop_cm:
                # gathers: fT (transpose mode), combined row (feats+coords),
                # query row
                GCH = 1024  # indices per dma_gather call
                NCH = CAP // GCH             # chunks
                BPC = GCH // 128             # blocks per chunk
                qsb = gpool.tile([128, NBLK, 4], F32, tag="qsb")
                nc.sync.dma_start(qsb[:], qsel_in[:].rearrange(
                    "a (b c) -> a b c", c=4))
                ssb_l, ftg_l, kwt_l = [], [], []
                for g in range(NCH):
                    ssb = gpool.tile([128, BPC, 64], F32, tag=f"ssb{g}")
                    if "ssb" in sk:
                        nc.vector.memset(ssb[:], 0.0)
                    else:
                        nc.gpsimd.dma_gather(
                            ssb[:], ftab_in[:].bitcast(F32),
                            midx_t[:, g * GCH // 16:(g + 1) * GCH // 16],
                            GCH, greg, 64, queue_num=swq())
                    ssb_l.append(ssb)
                    # fT via PE transpose, one 128-entry block at a time
                    ftg = gpool.tile([128, GCH], F16, tag=f"ftg{g}")
                    ssb16 = ssb[:].bitcast(F16)      # [128, BPC, 128]
                    for cc in range(BPC):
                        psumT = pstpool.tile([64, 128], F16, tag="pst")
                        nc.tensor.transpose(
                            psumT[:],
                            bass.AP(ssb16.tensor, ssb16.offset + cc * 128,
                                    [ssb16.ap[0], [1, 64]]),
                            ident_t[:])
                        nc.scalar.copy(
                            ap_part(ftg[:], 0, 64, cc * 128, [[1, 128]]),
                            psumT[:])
                    ftg_l.append(ftg)
                    # rel = s - q; kw = relu(1 - sqrt(d2)/sigma)
                    rel = gpool.tile([128, BPC, 3], F32, tag=f"rel{g}")
                    nc.vector.tensor_tensor(
                        out=rel[:],
                        in0=ap_view(ssb[:], 32, [[64, BPC], [1, 3]]),
                        in1=ap_view(qsb[:], g * BPC * 4, [[4, BPC], [1, 3]]),
                        op=mybir.AluOpType.subtract)
                    diff = gpool.tile([128, BPC * 45], F32, tag=f"diff{g}")
                    nc.vector.tensor_tensor(
                        out=ap_view(diff[:], 0, [[45, BPC], [3, P], [1, 3]]),
                        in0=ap_view(rel[:], 0, [[3, BPC], [0, P], [1, 3]]),
                        in1=ap_view(kpcat_t[:], 0, [[0, BPC], [3, P], [1, 3]]),
                        op=mybir.AluOpType.subtract)
                    nc.scalar.activation(diff[:], diff[:],
                                         mybir.ActivationFunctionType.Square,
                                         bias=0.0, scale=1.0)
                    kwt = gpool.tile([128, BPC, 16], F32, tag=f"kw{g}")
                    nc.vector.memset(kwt[:], 0.0)
                    nc.vector.tensor_reduce(
                        out=ap_view(kwt[:], 0, [[16, BPC], [1, P]]),
                        in_=ap_view(diff[:], 0, [[45, BPC], [3, P], [1, 3]]),
                        axis=mybir.AxisListType.X, op=mybir.AluOpType.add)
                    nc.scalar.activation(kwt[:], kwt[:],
                                         mybir.ActivationFunctionType.Sqrt,
                                         bias=kpcat_t[:, 45:46], scale=1.0)
                    nc.scalar.activation(kwt[:], kwt[:],
                                         mybir.ActivationFunctionType.Relu,
                                         bias=1.0, scale=kpcat_t[:, 46:47])
                    kwt_l.append(kwt)

                scat = gpool.tile([128, NBLK, C_OUT], F32, tag="scat")
                lp = nc.allow_low_precision(
                    reason="f16 weighted intermediates; validated vs "
                           "reference at 3e-4 rel err")
                lp.__enter__()
                for c in range(NBLK):
                    wtdm = wpool.tile([128, 2048], F16, tag="wtdm")
                    for hw in range(2):  # p 0:8 | p 8:16 (slot 15 zero-W)
                        psumF = psfpool.tile([128, 1024], F32, tag="psf")
                        for k in range(2):
                            nc.tensor.matmul(
                                psumF[:, k * 512:(k + 1) * 512],
                                ap_part(ftg_l[c // BPC][:], 0, C_IN,
                                        (c % BPC) * 128, [[1, 128]]),
                                ap_part(wcat_t[:], 0, C_IN,
                                        hw * 1024 + k * 512, [[1, 512]]),
                                start=True, stop=True)
                        nc.vector.tensor_tensor(
                            out=ap_view(wtdm[:], hw * 1024,
                                        [[C_OUT, 8], [1, C_OUT]]),
                            in0=ap_view(psumF[:], 0, [[C_OUT, 8], [1, C_OUT]]),
                            in1=ap_view(kwt_l[c // BPC][:],
                                        (c % BPC) * 16 + hw * 8,
                                        [[1, 8], [0, C_OUT]]),
                            op=mybir.AluOpType.mult)
                    tr1 = wpool.tile([128, 1024], F16, tag="tr1")
                    nc.vector.tensor_tensor(
                        out=tr1[:], in0=wtdm[:, 0:1024], in1=wtdm[:, 1024:2048],
                        op=mybir.AluOpType.add)
                    tr2 = wpool.tile([128, 512], F16, tag="tr2")
                    nc.vector.tensor_tensor(
                        out=tr2[:], in0=tr1[:, 0:512], in1=tr1[:, 512:1024],
                        op=mybir.AluOpType.add)
                    psum2 = ps2pool.tile([128, C_OUT], F32, tag="ps2")
                    for s in range(4):
                        nc.tensor.matmul(
                            psum2[:], seg_t[:, c * 128:(c + 1) * 128],
                            tr2[:, s * 128:(s + 1) * 128],
                            start=(s == 0), stop=(s == 3))
                    nc.scalar.copy(
                        ap_view(scat[:], c * C_OUT, [[1, C_OUT]]), psum2[:])

                lp.__exit__(None, None, None)
                if "scatter" not in sk:
                    qcnt = [0, 0, 0, 0]
                    for g in range(CAP // GCH):
                        q = 0
                        nc.gpsimd.dma_scatter_add(
                            out_t[:],
                            ap_view(scat[:], g * (GCH // 128) * C_OUT,
                                    [[C_OUT, GCH // 128], [1, C_OUT]]),
                            nscidx_t[:, g * GCH // 16:(g + 1) * GCH // 16],
                            GCH, greg, C_OUT,
                            queue_num=q).then_inc(done_sems[q], 16)
                        qcnt[q] += 16
                    for q in range(4):
                        if qcnt[q]:
                            nc.gpsimd.wait_ge(done_sems[q], qcnt[q])
                else:
                    nc.sync.dma_start(out_t[0:128, :],
                                      ap_view(scat[:], 0, [[1, C_OUT]]))
    return nc


def _make_runner_sparse(nc, n_cores):
    bass2jax.install_neuronx_cc_hook()
    from jax.sharding import Mesh, PartitionSpec
    from jax.experimental.shard_map import shard_map

    partition_name = nc.partition_id_tensor.name if nc.partition_id_tensor else None
    in_names, out_names, out_avals = [], [], []
    for alloc in nc.m.functions[0].allocations:
        if not isinstance(alloc, mybir.MemoryLocationSet):
            continue
        name = alloc.memorylocations[0].name
        if alloc.kind == "ExternalInput":
            if name != partition_name:
                in_names.append(name)
        elif alloc.kind == "ExternalOutput":
            shape = tuple(alloc.tensor_shape)
            dtype = mybir.dt.np(alloc.dtype)
            out_names.append(name)
            out_avals.append(jax.core.ShapedArray(shape, dtype))
    n_params = len(in_names)
    n_outs = len(out_avals)
    all_in = in_names + out_names + ([partition_name] if partition_name else [])

    def _body(*args):
        operands = list(args)
        if partition_name is not None:
            operands.append(bass2jax.partition_id_tensor())
        outs = bass2jax._bass_exec_p.bind(
            *operands, out_avals=tuple(out_avals), in_names=tuple(all_in),
            out_names=tuple(out_names), lowering_input_output_aliases=(),
            sim_require_finite=False, sim_require_nnan=False, nc=nc)
        return tuple(outs)

    devices = jax.devices()[:n_cores]
    mesh = Mesh(np.asarray(devices), ("core",))
    in_specs = (PartitionSpec("core"),) * (n_params + n_outs)
    out_specs = (PartitionSpec("core"),) * n_outs
    donate = tuple(range(n_params, n_params + n_outs))
    jit_fn = jax.jit(
        shard_map(_body, mesh=mesh, in_specs=in_specs, out_specs=out_specs,
                  check_rep=False), donate_argnums=donate, keep_unused=True)

    def run(in_maps, out_prefills):
        per_core = [[np.asarray(m[n]) for n in in_names] for m in in_maps]
        args = [np.concatenate([per_core[c][i] for c in range(n_cores)], axis=0)
                for i in range(n_params)]
        args += [np.concatenate([np.asarray(p[n]) for p in out_prefills], axis=0)
                 for n in out_names]
        outs = [np.asarray(o) for o in jit_fn(*args)]
        return [{n: outs[i].reshape(n_cores, *out_avals[i].shape)[c]
                 for i, n in enumerate(out_names)}
                for c in range(n_cores)], jit_fn, args

    return run


def _get_runner_sparse():
    if "sparse" not in _BUILT:
        nc = build_sparse()
        _BUILT["sparse"] = _make_runner_sparse(nc, N_CORES)
    return _BUILT["sparse"]


def _wrap16(vals, pad_val, dtype=np.int16):
    """List -> [128, CAP//16] wrapped (entry j at [j%16, j//16]), replicated
    across the 8 gpsimd cores."""
    buf = np.full(CAP, pad_val, dtype)
    buf[:len(vals)] = vals
    w = buf.reshape(CAP // 16, 16).T          # [16, CAP//16]
    return np.ascontiguousarray(np.tile(w, (8, 1)))


def _host_prep_sparse(qp, sp, sf, ni, w, bias_v, kpv):
    """Returns (in_maps, out_prefills) or None if candidates exceed CAP."""
    # conservative candidate radius: a hit needs |s - q| < sigma + max|kp|
    rmax = SIGMA + float(np.sqrt((kpv * kpv).sum(axis=1)).max())
    t_cell = (rmax * GRID + math.sqrt(3.0)) ** 2
    scell = np.clip((sp * GRID).astype(np.int32), 0, GRID - 1)
    qcell = np.clip((qp * GRID).astype(np.int32), 0, GRID - 1)

    wcat = np.zeros((128, 2048), np.float16)
    wcat[:C_IN, :P * C_OUT] = (np.transpose(w, (1, 0, 2)) / 16.0
                               ).reshape(C_IN, -1)
    wcat[C_IN:] = wcat[:C_IN]
    kpcat = np.zeros((128, 48), np.float32)
    kpcat[:, :45] = kpv.reshape(1, 45)
    kpcat[:, 45] = 1e-10
    kpcat[:, 46] = -1.0 / SIGMA

    in_maps, out_prefills = [], []
    for c in range(N_CORES):
        b, half = divmod(c, 2)
        n0 = half * NQ_CORE
        nib = ni[b, n0:n0 + NQ_CORE]
        dc = scell[b][nib] - qcell[b, n0:n0 + NQ_CORE, None, :]
        d2 = (dc.astype(np.int64) ** 2).sum(axis=2)
        nn, kk = np.nonzero(d2 <= t_cell)      # sorted by n (row-major)
        mm = nib[nn, kk]

        # pack into 128-entry blocks; no query group spans a block boundary
        m_list = np.zeros(CAP, np.int16)
        n_list = np.zeros(CAP, np.int16)
        seg = np.zeros((128, CAP), np.float16)
        sc_idx = np.full(CAP, TRASH, np.int16)
        uniq, counts = np.unique(nn, return_counts=True)
        t = 0            # global entry cursor
        gi = 0           # group cursor
        ok = True
        off = 0          # start of each group's pairs in nn/kk
        for g in range(len(uniq)):
            cnt = counts[g]
            blk, pos = divmod(t, 128)
            if pos + cnt > 128:                # pad to next block
                t = (blk + 1) * 128
                blk, pos = blk + 1, 0
            if t + cnt > CAP:
                ok = False
                break
            d = 127                            # d-slot for this group
            # d slots allocated in order of first use within the block
            # (track per-block next free slot)
            m_list[t:t + cnt] = mm[off:off + cnt]
            n_list[t:t + cnt] = uniq[g]
            t += cnt
            off += cnt
        if not ok:
            return None
        # second pass: assign d-slots and seg/sc_idx now that layout is fixed
        seg[:] = 0
        sc_idx[:] = TRASH
        blk_next = np.zeros(NBLK, np.int32)
        t = 0
        off = 0
        for g in range(len(uniq)):
            cnt = counts[g]
            blk, pos = divmod(t, 128)
            if pos + cnt > 128:
                t = (blk + 1) * 128
                blk, pos = blk + 1, 0
            d = blk_next[blk]
            blk_next[blk] += 1
            seg[pos:pos + cnt, blk * 128 + d] = 1.0
            sc_idx[blk * 128 + d] = uniq[g]
            t += cnt
            off += cnt
        # pad entries (between groups / tail): m=0, n=0 gathers; their seg
        # column stays 0 -> contribute nothing; unused d-slots scatter to
        # TRASH row.

        ftab = np.zeros((M, 128), np.float16)
        ftab[:, :C_IN] = sf[b]
        ftab.view(np.float32)[:, 32:35] = sp[b]
        qsel = np.zeros((CAP, 4), np.float32)
        qsel[:, :3] = qp[b, n0 + n_list.astype(np.int64)]
        qsel = np.ascontiguousarray(
            qsel.reshape(NBLK, 128, 4).transpose(1, 0, 2)).reshape(128, -1)
        in_maps.append({
            "ftab": ftab, "qsel": qsel, "wcat": wcat, "kpcat": kpcat,
            "seg": seg, "ident16": np.eye(128, dtype=np.float16),
            "midx": _wrap16(m_list, 0),
            "nscidx": _wrap16(sc_idx, TRASH),
        })
        out_prefills.append({
            "out": np.tile(bias_v.reshape(1, C_OUT),
                           (NQ_CORE + 1, 1)).astype(np.float32)})
    return in_maps, out_prefills


def _kernel_dense(qp_raw, sp_raw, sf_raw, ni_raw, w_raw, bias_raw, kp_raw):
    kp = np.asarray(kp_raw, np.float32)
    run = _get_runner(kp)
    in_maps = _host_prep(qp_raw, sp_raw, sf_raw, ni_raw, w_raw, bias_raw,
                         kp_raw)
    results, _, _ = run(in_maps)
    out = np.zeros((B, N, C_OUT), np.float32)
    for c in range(N_CORES):
        b, half = divmod(c, 2)
        n0 = half * NQ_CORE
        out[b, n0:n0 + NQ_CORE, :] = results[c]["out"]
    return out


def kernel(query_points, support_points, support_features, neighbor_indices,
           weights, bias, kernel_points):
    qp = np.asarray(query_points, np.float32)
    sp = np.asarray(support_points, np.float32)
    sf = np.asarray(support_features, np.float32)
    ni = np.clip(np.asarray(neighbor_indices), 0, M - 1).astype(np.int32)
    w = np.asarray(weights, np.float32)
    bias_v = np.asarray(bias, np.float32)
    kpv = np.asarray(kernel_points, np.float32)

    prep = _host_prep_sparse(qp, sp, sf, ni, w, bias_v, kpv)
    if prep is None:
        return _kernel_dense(query_points, support_points, support_features,
                             neighbor_indices, weights, bias, kernel_points)
    in_maps, out_prefills = prep
    run = _get_runner_sparse()
    results, _, _ = run(in_maps, out_prefills)
    out = np.zeros((B, N, C_OUT), np.float32)
    for c in range(N_CORES):
        b, half = divmod(c, 2)
        n0 = half * NQ_CORE
        out[b, n0:n0 + NQ_CORE, :] = results[c]["out"][:NQ_CORE]

    # exact neighbor-count correction (reference divides by the number of
    # neighbors with nonzero features, clipped to >= 1; the device divides
    # by K=16). For randn features every row is nonzero, so cnt == 16 and
    # this is a no-op; handle degenerate inputs on host for full fidelity.
    row_nz = np.abs(sf).sum(axis=2) > 0          # [B, M]
    if not row_nz.all():
        z = row_nz.astype(np.float32)
        cnt = np.clip(
            z[np.arange(B)[:, None, None], ni].sum(axis=2), 1.0, None)
        out = (out - bias_v) * (16.0 / cnt)[..., None] + bias_v
    return out

